# revision 11
# baseline (speedup 1.0000x reference)
"""MoE top-1 routing kernel for 8 TRN2 NeuronCores (expert parallelism).

Self-contained: takes full inputs, shards experts across 8 cores, returns the
full output. Routing (softmax gate, argmax, capacity-ordered positions) is done
on-device; each core gathers its expert's tokens, runs the FFN in bf16, and
scatters gate-scaled rows into a private zero-initialized output; the host sums
the 8 disjoint partials (top-1 routing makes them disjoint).

Position computation: per-128-token-tile inclusive prefix via a triangular
matmul, tile offsets via one tensor_tensor_scan over the 64 tile sums, then the
token->slot map is inverted into a BIG-prefilled DRAM index array with per-tile
indirect scatters (OOB indices silently dropped => capacity drop for free).
"""
import numpy as np
import ml_dtypes
from contextlib import ExitStack

import concourse.bass as bass
import concourse.tile as tile
from concourse import bacc, mybir
from concourse.bass_utils import run_bass_kernel_spmd

dt = mybir.dt

B, S, M, E, DFF = 4, 2048, 1024, 8, 4096
T = B * S                  # 8192 tokens
C = int(1.25 * T / E)      # 1280 capacity
P = 128
NT = T // P                # 64 token tiles
MC = M // P                # 8 m chunks
DC = DFF // P              # 32 dff chunks
SCN = C // P               # 10 slot chunks
HALF = C // 2              # 640 slots per half
BIG = 1.0e9
BIGI = 2 ** 30

_CACHE = {}


def _build_nc(stage=5):
    nc = bacc.Bacc("TRN2", target_bir_lowering=False, debug=False)

    # ---- I/O ----
    xT = nc.dram_tensor("xT", [M, T], dt.float32, kind="ExternalInput")
    xb = nc.dram_tensor("xb", [T, M], dt.bfloat16, kind="ExternalInput")
    wg = nc.dram_tensor("wg", [M, E], dt.float32, kind="ExternalInput")
    w1p = nc.dram_tensor("w1p", [DC, P, MC, P], dt.bfloat16, kind="ExternalInput")
    w2p = nc.dram_tensor("w2p", [P, DC, M], dt.bfloat16, kind="ExternalInput")
    b1v = nc.dram_tensor("b1v", [DFF], dt.float32, kind="ExternalInput")
    b2b = nc.dram_tensor("b2b", [P, M], dt.float32, kind="ExternalInput")
    ecol = nc.dram_tensor("ecol", [P, 1], dt.float32, kind="ExternalInput")
    tokids = nc.dram_tensor("tokids", [P, NT], dt.int32, kind="ExternalInput")
    triu = nc.dram_tensor("triu", [P, P], dt.float32, kind="ExternalInput")
    identb = nc.dram_tensor("identb", [P, P], dt.bfloat16, kind="ExternalInput")
    outd = nc.dram_tensor("out", [T, M], dt.float32, kind="ExternalOutput")

    # ---- internal DRAM ----
    idxd = nc.dram_tensor("idxd", [C, 1], dt.int32)
    gate_d = nc.dram_tensor("gate_d", [T, 1], dt.float32)

    with tile.TileContext(nc) as tc, ExitStack() as ctx:
        sb = ctx.enter_context(tc.tile_pool(name="sb", bufs=1))
        sbx = ctx.enter_context(tc.tile_pool(name="sbx", bufs=12))  # x stream
        sbw1 = ctx.enter_context(tc.tile_pool(name="sbw1", bufs=3))  # w1 stream
        sbg = ctx.enter_context(tc.tile_pool(name="sbg", bufs=3))   # gather tiles
        sbst = ctx.enter_context(tc.tile_pool(name="sbst", bufs=3))  # staging
        sbr = ctx.enter_context(tc.tile_pool(name="sbr", bufs=4))   # routing small

        # ---------- persistent tiles ----------
        wgt = sb.tile([P, MC * E], dt.float32)       # gate weights (mc, e)
        nc.sync.dma_start(wgt[:], wg[:].rearrange("(mc p) e -> p mc e", p=P))
        b1t = sb.tile([P, DC], dt.float32)           # b1 per-partition cols
        nc.sync.dma_start(b1t[:], b1v[:].rearrange("(d p) -> p d", p=P))
        b2t = sb.tile([P, M], dt.float32)
        nc.sync.dma_start(b2t[:], b2b[:])
        ect = sb.tile([P, 1], dt.float32)
        nc.sync.dma_start(ect[:], ecol[:])
        tokt = sb.tile([P, NT], dt.int32)
        nc.sync.dma_start(tokt[:], tokids[:])
        trit = sb.tile([P, P], dt.float32)
        nc.sync.dma_start(trit[:], triu[:])
        idb = sb.tile([P, P], dt.bfloat16)
        nc.sync.dma_start(idb[:], identb[:])
        ones1 = sb.tile([1, P], dt.float32)
        nc.gpsimd.memset(ones1[:], 1.0)
        onescol = sb.tile([P, 1], dt.float32)
        nc.gpsimd.memset(onescol[:], 1.0)
        bigcol = sb.tile([P, 1], dt.float32)
        nc.gpsimd.memset(bigcol[:], BIG)
        bigti = sb.tile([P, SCN], dt.int32)
        nc.gpsimd.memset(bigti[:], BIGI)
        w2t = sb.tile([P, DC * M], dt.bfloat16)      # resident w2 (d, m)
        if stage >= 5:
            for q in range(4):
                nc.sync.dma_start(
                    w2t[:, q * 8 * M:(q + 1) * 8 * M],
                    w2p[:, q * 8:(q + 1) * 8, :])

        mine_stk = sb.tile([P, NT], dt.float32)
        gate_stk = sb.tile([P, NT], dt.float32)
        idx_t = sb.tile([P, SCN], dt.int32)
        gate_f = sb.tile([P, SCN], dt.float32)

        # prefill idx DRAM with BIG so empty slots are skipped everywhere
        nc.sync.dma_start(idxd[:].rearrange("(c p) one -> p c one", p=P), bigti[:])

        # ---------- phase A: gating ----------
        with tc.tile_pool(name="psg", bufs=4, space="PSUM") as psg, \
             tc.tile_pool(name="pts", bufs=1, space="PSUM") as pts:
            ptsum = pts.tile([1, NT], dt.float32)
            for tb in range(8):          # token blocks of 1024
                xts = []
                for k in range(MC):
                    xt = sbx.tile([P, 1024], dt.float32, tag="xt")
                    nc.sync.dma_start(
                        xt[:], xT[k * P:(k + 1) * P, tb * 1024:(tb + 1) * 1024])
                    xts.append(xt)
                for ti in range(8):      # 128-token tiles within block
                    t = tb * 8 + ti
                    pg = psg.tile([P, E], dt.float32, tag="pg")
                    for k in range(MC):
                        nc.tensor.matmul(
                            pg[:], lhsT=xts[k][:, ti * P:(ti + 1) * P],
                            rhs=wgt[:, k * E:(k + 1) * E],
                            start=(k == 0), stop=(k == MC - 1))
                    lg = sbr.tile([P, E], dt.float32, tag="lg")
                    nc.vector.tensor_copy(lg[:], pg[:])
                    mx = sbr.tile([P, E], dt.float32, tag="mx")
                    nc.vector.max(mx[:], lg[:])
                    mi = sbr.tile([P, E], dt.uint32, tag="mi")
                    nc.vector.max_index(mi[:], mx[:], lg[:])
                    ngm = sbr.tile([P, 1], dt.float32, tag="ngm")
                    nc.vector.tensor_scalar_mul(ngm[:], mx[:, 0:1], -1.0)
                    e8 = sbr.tile([P, E], dt.float32, tag="e8")
                    nc.scalar.activation(
                        e8[:], lg[:], mybir.ActivationFunctionType.Exp,
                        bias=ngm[:, 0:1], scale=1.0)
                    s1 = sbr.tile([P, 1], dt.float32, tag="s1")
                    nc.vector.reduce_sum(s1[:], e8[:], axis=mybir.AxisListType.X)
                    nc.vector.reciprocal(gate_stk[:, t:t + 1], s1[:])
                    ef = sbr.tile([P, 1], dt.float32, tag="ef")
                    nc.vector.tensor_copy(ef[:], mi[:, 0:1])
                    nc.vector.tensor_tensor(
                        out=mine_stk[:, t:t + 1], in0=ef[:], in1=ect[:],
                        op=mybir.AluOpType.is_equal)
                    # per-tile count of mine tokens -> ptsum[0, t]
                    nc.tensor.matmul(
                        ptsum[0:1, t:t + 1], lhsT=mine_stk[:, t:t + 1],
                        rhs=onescol[:], start=True, stop=True)

        if stage < 2:
            nc.compile()
            return nc

        # ---------- phase B: positions + token->slot inversion ----------
        tsums = sb.tile([1, NT], dt.float32)
        nc.vector.tensor_copy(tsums[:], ptsum[:])
        zrow = sb.tile([1, NT], dt.float32)
        nc.gpsimd.memset(zrow[:], 0.0)
        incl = sb.tile([1, NT], dt.float32)
        nc.vector.tensor_tensor_scan(
            out=incl[:], data0=tsums[:], data1=zrow[:], initial=0.0,
            op0=mybir.AluOpType.add, op1=mybir.AluOpType.add)
        offs = sb.tile([1, NT], dt.float32)
        nc.vector.tensor_tensor(
            out=offs[:], in0=incl[:], in1=tsums[:],
            op=mybir.AluOpType.subtract)

        with tc.tile_pool(name="ppos", bufs=4, space="PSUM") as ppos:
            for t in range(NT):
                pp = ppos.tile([P, 1], dt.float32, tag="pp")
                nc.tensor.matmul(
                    pp[:], lhsT=trit[:], rhs=mine_stk[:, t:t + 1],
                    start=True, stop=False)
                nc.tensor.matmul(
                    pp[:], lhsT=ones1[:], rhs=offs[0:1, t:t + 1],
                    start=False, stop=True)
                slotm1 = sbr.tile([P, 1], dt.float32, tag="slotm1")
                nc.vector.tensor_scalar_add(slotm1[:], pp[:], -1.0)
                mu8 = sbr.tile([P, 1], dt.uint8, tag="mu8")
                nc.vector.tensor_scalar(
                    out=mu8[:], in0=mine_stk[:, t:t + 1], scalar1=0.5,
                    scalar2=None, op0=mybir.AluOpType.is_gt)
                slotf = sbr.tile([P, 1], dt.float32, tag="slotf")
                nc.vector.select(slotf[:], mu8[:], slotm1[:], bigcol[:])
                sloti = sbr.tile([P, 1], dt.int32, tag="sloti")
                nc.vector.tensor_copy(sloti[:], slotf[:])
                nc.gpsimd.indirect_dma_start(
                    out=idxd[:], out_offset=bass.IndirectOffsetOnAxis(
                        ap=sloti[:, 0:1], axis=0),
                    in_=tokt[:, t:t + 1], in_offset=None,
                    bounds_check=C - 1, oob_is_err=False)

        # gate per token -> DRAM, then gather per slot
        nc.sync.dma_start(
            gate_d[:].rearrange("(t p) one -> p t one", p=P), gate_stk[:])
        nc.sync.dma_start(idx_t[:], idxd[:].rearrange("(c p) one -> p c one", p=P))
        for sc in range(SCN):
            gc = sbr.tile([P, 1], dt.float32, tag="gc")
            nc.gpsimd.memset(gc[:], 0.0)
            nc.gpsimd.indirect_dma_start(
                out=gc[:], out_offset=None, in_=gate_d[:],
                in_offset=bass.IndirectOffsetOnAxis(
                    ap=idx_t[:, sc:sc + 1], axis=0),
                bounds_check=T - 1, oob_is_err=False)
            nc.vector.tensor_copy(gate_f[:, sc:sc + 1], gc[:])

        if stage < 3:
            nc.compile()
            return nc

        # ---------- phases C/D/E per half ----------
        with (
            tc.tile_pool(name="pstr", bufs=2, space="PSUM") as pstr,
            tc.tile_pool(name="ps1", bufs=2, space="PSUM") as ps1,
            tc.tile_pool(name="ps2", bufs=2, space="PSUM") as ps2,
        ):
            for h in range(2):
                dispT = sb.tile([P, MC * HALF], dt.bfloat16, tag="dispT")
                hT = sb.tile([P, DC * HALF], dt.bfloat16, tag="hT")
                # dispatch: gather + transpose
                for s5 in range(5):
                    sc = h * 5 + s5
                    gx = sbg.tile([P, M], dt.bfloat16, tag="gx")
                    nc.gpsimd.memset(gx[:], 0.0)
                    nc.gpsimd.indirect_dma_start(
                        out=gx[:], out_offset=None, in_=xb[:],
                        in_offset=bass.IndirectOffsetOnAxis(
                            ap=idx_t[:, sc:sc + 1], axis=0),
                        bounds_check=T - 1, oob_is_err=False)
                    for mm in range(MC):
                        ptr = pstr.tile([P, P], dt.bfloat16, tag="ptr")
                        nc.tensor.transpose(
                            out=ptr[:], in_=gx[:, mm * P:(mm + 1) * P],
                            identity=idb[:])
                        nc.vector.tensor_copy(
                            dispT[:, mm * HALF + s5 * P:
                                  mm * HALF + (s5 + 1) * P],
                            ptr[:])
                # FFN1
                if stage >= 4:
                    for d in range(DC):
                        w1t = sbw1.tile([P, M], dt.bfloat16, tag="w1t")
                        nc.sync.dma_start(w1t[:], w1p[d])
                        pA = ps1.tile([P, 512], dt.float32, tag="pA")
                        pB = ps1.tile([P, P], dt.float32, tag="pB")
                        for mc in range(MC):
                            lhs = w1t[:, mc * P:(mc + 1) * P]
                            nc.tensor.matmul(
                                pA[:], lhsT=lhs,
                                rhs=dispT[:, mc * HALF:mc * HALF + 512],
                                start=(mc == 0), stop=(mc == MC - 1))
                            nc.tensor.matmul(
                                pB[:], lhsT=lhs,
                                rhs=dispT[:, mc * HALF + 512:(mc + 1) * HALF],
                                start=(mc == 0), stop=(mc == MC - 1))
                        nc.scalar.activation(
                            hT[:, d * HALF:d * HALF + 512], pA[:],
                            mybir.ActivationFunctionType.Relu,
                            bias=b1t[:, d:d + 1], scale=1.0)
                        nc.scalar.activation(
                            hT[:, d * HALF + 512:(d + 1) * HALF], pB[:],
                            mybir.ActivationFunctionType.Relu,
                            bias=b1t[:, d:d + 1], scale=1.0)
                # FFN2 + combine + scatter
                if stage >= 5:
                    for s5 in range(5):
                        sc = h * 5 + s5
                        st = sbst.tile([P, M], dt.float32, tag="st")
                        for mm in range(2):
                            po = ps2.tile([P, 512], dt.float32, tag="po")
                            for d in range(DC):
                                nc.tensor.matmul(
                                    po[:],
                                    lhsT=hT[:, d * HALF + s5 * P:
                                            d * HALF + (s5 + 1) * P],
                                    rhs=w2t[:, d * M + mm * 512:
                                            d * M + (mm + 1) * 512],
                                    start=(d == 0), stop=(d == DC - 1))
                            nc.vector.tensor_tensor(
                                out=st[:, mm * 512:(mm + 1) * 512], in0=po[:],
                                in1=b2t[:, mm * 512:(mm + 1) * 512],
                                op=mybir.AluOpType.add)
                        nc.vector.tensor_scalar_mul(
                            st[:], st[:], gate_f[:, sc:sc + 1])
                        nc.gpsimd.indirect_dma_start(
                            out=outd[:], out_offset=bass.IndirectOffsetOnAxis(
                                ap=idx_t[:, sc:sc + 1], axis=0),
                            in_=st[:], in_offset=None,
                            bounds_check=T - 1, oob_is_err=False)

    nc.compile()
    return nc


def _prep_inputs(x, wg, w1, b1, w2, b2):
    bf16 = ml_dtypes.bfloat16
    tokens = np.ascontiguousarray(x.reshape(T, M)).astype(np.float32)
    xT = np.ascontiguousarray(tokens.T)
    xb = tokens.astype(bf16)
    wgf = np.ascontiguousarray(wg.astype(np.float32))
    tokids = np.arange(T, dtype=np.int32).reshape(NT, P).T.copy()
    triu = np.triu(np.ones((P, P), dtype=np.float32))
    identb = np.eye(P).astype(bf16)
    in_maps = []
    for e in range(E):
        w1e = np.ascontiguousarray(w1[e]).astype(bf16)          # [M, DFF]
        w1pk = np.ascontiguousarray(
            w1e.reshape(MC, P, DC, P).transpose(2, 1, 0, 3))    # [DC,P,MC,P]
        w2e = np.ascontiguousarray(w2[e]).astype(bf16)          # [DFF, M]
        w2pk = np.ascontiguousarray(
            w2e.reshape(DC, P, M).transpose(1, 0, 2))           # [P,DC,M]
        in_maps.append({
            "xT": xT, "xb": xb, "wg": wgf,
            "w1p": w1pk, "w2p": w2pk,
            "b1v": np.ascontiguousarray(b1[e]).astype(np.float32),
            "b2b": np.tile(np.asarray(b2[e], dtype=np.float32), (P, 1)),
            "ecol": np.full((P, 1), float(e), dtype=np.float32),
            "tokids": tokids, "triu": triu, "identb": identb,
        })
    return in_maps


def kernel(x, wg, w1, b1, w2, b2, _trace=False):
    if "nc" not in _CACHE:
        _CACHE["nc"] = _build_nc()
    nc = _CACHE["nc"]
    in_maps = _prep_inputs(
        np.asarray(x), np.asarray(wg), np.asarray(w1),
        np.asarray(b1), np.asarray(w2), np.asarray(b2))
    res = run_bass_kernel_spmd(nc, in_maps, list(range(E)), trace=_trace)
    _CACHE["last_results"] = res
    full = np.zeros((T, M), dtype=np.float32)
    for e in range(E):
        full += res.results[e]["out"]
    return full.reshape(B, S, M)


# revision 13
# speedup vs baseline: 1.2480x; 1.2480x over previous
"""MoE top-1 routing kernel for 8 TRN2 NeuronCores (expert parallelism).

Self-contained: takes full inputs, shards experts across 8 cores, returns the
full output (host sums the 8 disjoint per-expert partials).

v2 design:
- Gating is token-sharded: core j computes fp32 logits/argmax/gate for its
  1024 tokens only, then an AllGather shares the [T, 2] (expert_id, gate)
  metadata with every core.
- Positions: batched triangular-matmul prefix (one 128x64 matmul) + one
  tensor_tensor_scan over tile sums; token->slot map inverted into a
  BIG-prefilled DRAM [C, 2] (token_id, gate) array via 64 row-scatters
  (OOB silently dropped => capacity drop for free).
- FFN in bf16 (w2 resident, w1 streamed twice over two 640-slot halves),
  fused bias+ReLU on the scalar engine, gate-scaled rows scattered into a
  pre-zeroed private output.
"""
import numpy as np
import ml_dtypes
from contextlib import ExitStack

import concourse.bass as bass
import concourse.tile as tile
from concourse import bacc, mybir
from concourse.bass_utils import run_bass_kernel_spmd

dt = mybir.dt

B, S, M, E, DFF = 4, 2048, 1024, 8, 4096
T = B * S                  # 8192 tokens
C = int(1.25 * T / E)      # 1280 capacity
P = 128
NT = T // P                # 64 token tiles
MC = M // P                # 8 m chunks
DC = DFF // P              # 32 dff chunks
SCN = C // P               # 10 slot chunks
HALF = C // 2              # 640 slots per half
TSH = T // E               # 1024 tokens per shard
BIG = 1.0e9
BIGI = 2 ** 30

_CACHE = {}


def _build_nc(stage=5):
    nc = bacc.Bacc("TRN2", target_bir_lowering=False, debug=False)

    # ---- I/O ----
    xTs = nc.dram_tensor("xTs", [M, TSH], dt.float32, kind="ExternalInput")
    xb = nc.dram_tensor("xb", [T, M], dt.bfloat16, kind="ExternalInput")
    wg = nc.dram_tensor("wg", [M, E], dt.float32, kind="ExternalInput")
    w1p = nc.dram_tensor("w1p", [DC, P, MC, P], dt.bfloat16, kind="ExternalInput")
    w2p = nc.dram_tensor("w2p", [P, DC, M], dt.bfloat16, kind="ExternalInput")
    b1v = nc.dram_tensor("b1v", [DFF], dt.float32, kind="ExternalInput")
    b2b = nc.dram_tensor("b2b", [P, M], dt.float32, kind="ExternalInput")
    ecol = nc.dram_tensor("ecol", [P, 1], dt.float32, kind="ExternalInput")
    toksf = nc.dram_tensor("toksf", [P, NT], dt.float32, kind="ExternalInput")
    triu = nc.dram_tensor("triu", [P, P], dt.float32, kind="ExternalInput")
    identf = nc.dram_tensor("identf", [P, P], dt.float32, kind="ExternalInput")
    identb = nc.dram_tensor("identb", [P, P], dt.bfloat16, kind="ExternalInput")
    pfill = nc.dram_tensor("pfill", [P, SCN, 2], dt.float32, kind="ExternalInput")
    outd = nc.dram_tensor("out", [T, M], dt.float32, kind="ExternalOutput")

    # ---- internal DRAM ----
    eg_loc = nc.dram_tensor("eg_loc", [TSH, 2], dt.float32)
    eg_all = nc.dram_tensor("eg_all", [T, 2], dt.float32, addr_space="Shared")
    igd = nc.dram_tensor("igd", [C, 2], dt.float32)

    with tile.TileContext(nc) as tc, ExitStack() as ctx:
        sb = ctx.enter_context(tc.tile_pool(name="sb", bufs=1))
        sbx = ctx.enter_context(tc.tile_pool(name="sbx", bufs=9))   # x stream
        sbw1 = ctx.enter_context(tc.tile_pool(name="sbw1", bufs=3))  # w1 stream
        sbg = ctx.enter_context(tc.tile_pool(name="sbg", bufs=3))   # gather tiles
        sbst = ctx.enter_context(tc.tile_pool(name="sbst", bufs=3))  # staging
        sbr = ctx.enter_context(tc.tile_pool(name="sbr", bufs=4))   # routing small

        # ---------- persistent tiles (small consts on sync queue) ----------
        wgt = sb.tile([P, MC * E], dt.float32)       # gate weights (mc, e)
        nc.sync.dma_start(wgt[:], wg[:].rearrange("(mc p) e -> p mc e", p=P))
        b1t = sb.tile([P, DC], dt.float32)           # b1 per-partition cols
        nc.sync.dma_start(b1t[:], b1v[:].rearrange("(d p) -> p d", p=P))
        ect = sb.tile([P, 1], dt.float32)
        nc.sync.dma_start(ect[:], ecol[:])
        tokf = sb.tile([P, NT], dt.float32)
        nc.sync.dma_start(tokf[:], toksf[:])
        trit = sb.tile([P, P], dt.float32)
        nc.sync.dma_start(trit[:], triu[:])
        idf = sb.tile([P, P], dt.float32)
        nc.sync.dma_start(idf[:], identf[:])
        idb = sb.tile([P, P], dt.bfloat16)
        nc.sync.dma_start(idb[:], identb[:])
        pft = sb.tile([P, SCN * 2], dt.float32)
        nc.sync.dma_start(pft[:], pfill[:])
        ones1 = sb.tile([1, P], dt.float32)
        nc.gpsimd.memset(ones1[:], 1.0)
        onescol = sb.tile([P, 1], dt.float32)
        nc.gpsimd.memset(onescol[:], 1.0)
        bigcol = sb.tile([P, NT], dt.float32)
        nc.gpsimd.memset(bigcol[:], BIG)
        # big tensors on the scalar (ACT) HWDGE queue so they don't block x
        b2t = sb.tile([P, M], dt.float32)
        nc.scalar.dma_start(b2t[:], b2b[:])
        w2t = sb.tile([P, DC * M], dt.bfloat16)      # resident w2 (d, m)
        if stage >= 5:
            for q in range(4):
                nc.scalar.dma_start(
                    w2t[:, q * 8 * M:(q + 1) * 8 * M],
                    w2p[:, q * 8:(q + 1) * 8, :])

        # prefill idx/gate DRAM: BIG ids (skipped everywhere), zero gates
        nc.sync.dma_start(igd[:].rearrange("(c p) two -> p c two", p=P), pft[:])

        eg_stk = sb.tile([P, 8 * 2], dt.float32)     # local (eidx, gate) cols
        idx_t = sb.tile([P, SCN], dt.int32)
        gate_f = sb.tile([P, SCN], dt.float32)

        # ---------- phase A: sharded gating over this core's 1024 tokens ---
        with tc.tile_pool(name="psg", bufs=4, space="PSUM") as psg:
            xts = []
            for k in range(MC):
                xt = sbx.tile([P, TSH], dt.float32, tag="xt")
                nc.sync.dma_start(xt[:], xTs[k * P:(k + 1) * P, :])
                xts.append(xt)
            for ti in range(TSH // P):   # 8 local tiles
                pg = psg.tile([P, E], dt.float32, tag="pg")
                for k in range(MC):
                    nc.tensor.matmul(
                        pg[:], lhsT=xts[k][:, ti * P:(ti + 1) * P],
                        rhs=wgt[:, k * E:(k + 1) * E],
                        start=(k == 0), stop=(k == MC - 1))
                lg = sbr.tile([P, E], dt.float32, tag="lg")
                nc.vector.tensor_copy(lg[:], pg[:])
                mx = sbr.tile([P, E], dt.float32, tag="mx")
                nc.vector.max(mx[:], lg[:])
                mi = sbr.tile([P, E], dt.uint32, tag="mi")
                nc.vector.max_index(mi[:], mx[:], lg[:])
                ngm = sbr.tile([P, 1], dt.float32, tag="ngm")
                nc.vector.tensor_scalar_mul(ngm[:], mx[:, 0:1], -1.0)
                e8 = sbr.tile([P, E], dt.float32, tag="e8")
                nc.scalar.activation(
                    e8[:], lg[:], mybir.ActivationFunctionType.Exp,
                    bias=ngm[:, 0:1], scale=1.0)
                s1 = sbr.tile([P, 1], dt.float32, tag="s1")
                nc.vector.reduce_sum(s1[:], e8[:], axis=mybir.AxisListType.X)
                nc.vector.reciprocal(eg_stk[:, 2 * ti + 1:2 * ti + 2], s1[:])
                nc.vector.tensor_copy(eg_stk[:, 2 * ti:2 * ti + 1], mi[:, 0:1])

        # share routing metadata with all cores
        nc.sync.dma_start(
            eg_loc[:].rearrange("(t p) two -> p t two", p=P), eg_stk[:])
        nc.gpsimd.collective_compute(
            "AllGather", mybir.AluOpType.bypass,
            ins=[eg_loc[:]], outs=[eg_all[:]],
            replica_groups=[list(range(E))])

        if stage < 2:
            nc.compile()
            return nc

        # ---------- phase B: batched positions + token->slot inversion -----
        eidx_stk = sb.tile([P, NT], dt.float32)
        gate_stk = sb.tile([P, NT], dt.float32)
        nc.sync.dma_start(
            eidx_stk[:],
            eg_all[:].rearrange("(t p) two -> p t two", p=P)[:, :, 0:1])
        nc.sync.dma_start(
            gate_stk[:],
            eg_all[:].rearrange("(t p) two -> p t two", p=P)[:, :, 1:2])
        mine_stk = sb.tile([P, NT], dt.float32)
        nc.vector.tensor_scalar(
            out=mine_stk[:], in0=eidx_stk[:], scalar1=ect[:, 0:1], scalar2=None,
            op0=mybir.AluOpType.is_equal)

        with tc.tile_pool(name="ppb", bufs=1, space="PSUM") as ppb:
            # tile sums -> [1, NT] row via matmul + transpose
            pts = ppb.tile([64, 1], dt.float32, tag="pts")
            nc.tensor.matmul(pts[:], lhsT=mine_stk[:], rhs=onescol[:],
                             start=True, stop=True)
            tcol = sb.tile([64, 1], dt.float32)
            nc.vector.tensor_copy(tcol[:], pts[:])
            ptr = ppb.tile([1, 64], dt.float32, tag="ptr")
            nc.tensor.transpose(out=ptr[:], in_=tcol[:], identity=idf[:64, :64])
            tsums = sb.tile([1, NT], dt.float32)
            nc.vector.tensor_copy(tsums[:], ptr[:])
            zrow = sb.tile([1, NT], dt.float32)
            nc.gpsimd.memset(zrow[:], 0.0)
            incl = sb.tile([1, NT], dt.float32)
            nc.vector.tensor_tensor_scan(
                out=incl[:], data0=tsums[:], data1=zrow[:], initial=0.0,
                op0=mybir.AluOpType.add, op1=mybir.AluOpType.add)
            offs = sb.tile([1, NT], dt.float32)
            nc.vector.tensor_tensor(
                out=offs[:], in0=incl[:], in1=tsums[:],
                op=mybir.AluOpType.subtract)
            # batched positions for all 64 tiles
            pall = ppb.tile([P, NT], dt.float32, tag="pall")
            nc.tensor.matmul(pall[:], lhsT=trit[:], rhs=mine_stk[:],
                             start=True, stop=False)
            nc.tensor.matmul(pall[:], lhsT=ones1[:], rhs=offs[:],
                             start=False, stop=True)
            slotm1 = sb.tile([P, NT], dt.float32)
            nc.vector.tensor_scalar_add(slotm1[:], pall[:], -1.0)
        mu8 = sb.tile([P, NT], dt.uint8)
        nc.vector.tensor_scalar(
            out=mu8[:], in0=mine_stk[:], scalar1=0.5, scalar2=None,
            op0=mybir.AluOpType.is_gt)
        slotf = sb.tile([P, NT], dt.float32)
        nc.vector.select(slotf[:], mu8[:], slotm1[:], bigcol[:])
        sloti = sb.tile([P, NT], dt.int32)
        nc.vector.tensor_copy(sloti[:], slotf[:])
        # (token_id, gate) pairs; interleaved columns
        pairs = sb.tile([P, NT * 2], dt.float32)
        nc.vector.tensor_copy(
            pairs[:].rearrange("p (t two) -> p t two", two=2)[:, :, 0:1],
            tokf[:])
        nc.vector.tensor_copy(
            pairs[:].rearrange("p (t two) -> p t two", two=2)[:, :, 1:2],
            gate_stk[:])
        for t in range(NT):
            nc.gpsimd.indirect_dma_start(
                out=igd[:], out_offset=bass.IndirectOffsetOnAxis(
                    ap=sloti[:, t:t + 1], axis=0),
                in_=pairs[:, 2 * t:2 * t + 2], in_offset=None,
                bounds_check=C - 1, oob_is_err=False)
        idxf = sb.tile([P, SCN], dt.float32)
        nc.sync.dma_start(
            idxf[:], igd[:].rearrange("(c p) two -> p c two", p=P)[:, :, 0:1])
        nc.sync.dma_start(
            gate_f[:], igd[:].rearrange("(c p) two -> p c two", p=P)[:, :, 1:2])
        nc.vector.tensor_copy(idx_t[:], idxf[:])

        if stage < 3:
            nc.compile()
            return nc

        # ---------- phases C/D/E per half ----------
        with (
            tc.tile_pool(name="pstr", bufs=2, space="PSUM") as pstr,
            tc.tile_pool(name="ps1", bufs=2, space="PSUM") as ps1,
            tc.tile_pool(name="ps2", bufs=2, space="PSUM") as ps2,
        ):
            for h in range(2):
                dispT = sb.tile([P, MC * HALF], dt.bfloat16, tag="dispT")
                hT = sb.tile([P, DC * HALF], dt.bfloat16, tag="hT")
                # dispatch: gather + transpose
                for s5 in range(5):
                    sc = h * 5 + s5
                    gx = sbg.tile([P, M], dt.bfloat16, tag="gx")
                    nc.gpsimd.memset(gx[:], 0.0)
                    nc.gpsimd.indirect_dma_start(
                        out=gx[:], out_offset=None, in_=xb[:],
                        in_offset=bass.IndirectOffsetOnAxis(
                            ap=idx_t[:, sc:sc + 1], axis=0),
                        bounds_check=T - 1, oob_is_err=False)
                    for mm in range(MC):
                        ptg = pstr.tile([P, P], dt.bfloat16, tag="ptg")
                        nc.tensor.transpose(
                            out=ptg[:], in_=gx[:, mm * P:(mm + 1) * P],
                            identity=idb[:])
                        nc.vector.tensor_copy(
                            dispT[:, mm * HALF + s5 * P:
                                  mm * HALF + (s5 + 1) * P],
                            ptg[:])
                # FFN1
                if stage >= 4:
                    for d in range(DC):
                        w1t = sbw1.tile([P, M], dt.bfloat16, tag="w1t")
                        nc.sync.dma_start(w1t[:], w1p[d])
                        pA = ps1.tile([P, 512], dt.float32, tag="pA")
                        pB = ps1.tile([P, P], dt.float32, tag="pB")
                        for mc in range(MC):
                            lhs = w1t[:, mc * P:(mc + 1) * P]
                            nc.tensor.matmul(
                                pA[:], lhsT=lhs,
                                rhs=dispT[:, mc * HALF:mc * HALF + 512],
                                start=(mc == 0), stop=(mc == MC - 1))
                            nc.tensor.matmul(
                                pB[:], lhsT=lhs,
                                rhs=dispT[:, mc * HALF + 512:(mc + 1) * HALF],
                                start=(mc == 0), stop=(mc == MC - 1))
                        nc.scalar.activation(
                            hT[:, d * HALF:d * HALF + 512], pA[:],
                            mybir.ActivationFunctionType.Relu,
                            bias=b1t[:, d:d + 1], scale=1.0)
                        nc.scalar.activation(
                            hT[:, d * HALF + 512:(d + 1) * HALF], pB[:],
                            mybir.ActivationFunctionType.Relu,
                            bias=b1t[:, d:d + 1], scale=1.0)
                # FFN2 + combine + scatter
                if stage >= 5:
                    for s5 in range(5):
                        sc = h * 5 + s5
                        st = sbst.tile([P, M], dt.float32, tag="st")
                        for mm in range(2):
                            po = ps2.tile([P, 512], dt.float32, tag="po")
                            for d in range(DC):
                                nc.tensor.matmul(
                                    po[:],
                                    lhsT=hT[:, d * HALF + s5 * P:
                                            d * HALF + (s5 + 1) * P],
                                    rhs=w2t[:, d * M + mm * 512:
                                            d * M + (mm + 1) * 512],
                                    start=(d == 0), stop=(d == DC - 1))
                            nc.vector.tensor_tensor(
                                out=st[:, mm * 512:(mm + 1) * 512], in0=po[:],
                                in1=b2t[:, mm * 512:(mm + 1) * 512],
                                op=mybir.AluOpType.add)
                        nc.vector.tensor_scalar_mul(
                            st[:], st[:], gate_f[:, sc:sc + 1])
                        nc.gpsimd.indirect_dma_start(
                            out=outd[:], out_offset=bass.IndirectOffsetOnAxis(
                                ap=idx_t[:, sc:sc + 1], axis=0),
                            in_=st[:], in_offset=None,
                            bounds_check=T - 1, oob_is_err=False)

    nc.compile()
    return nc


def _prep_inputs(x, wg, w1, b1, w2, b2):
    bf16 = ml_dtypes.bfloat16
    tokens = np.ascontiguousarray(x.reshape(T, M)).astype(np.float32)
    xT = np.ascontiguousarray(tokens.T)
    xb = tokens.astype(bf16)
    wgf = np.ascontiguousarray(wg.astype(np.float32))
    toksf = np.arange(T, dtype=np.float32).reshape(NT, P).T.copy()
    triu = np.triu(np.ones((P, P), dtype=np.float32))
    identf = np.eye(P, dtype=np.float32)
    identb = np.eye(P).astype(bf16)
    pfill = np.zeros((P, SCN, 2), dtype=np.float32)
    pfill[:, :, 0] = BIG
    in_maps = []
    for e in range(E):
        w1e = np.ascontiguousarray(w1[e]).astype(bf16)          # [M, DFF]
        w1pk = np.ascontiguousarray(
            w1e.reshape(MC, P, DC, P).transpose(2, 1, 0, 3))    # [DC,P,MC,P]
        w2e = np.ascontiguousarray(w2[e]).astype(bf16)          # [DFF, M]
        w2pk = np.ascontiguousarray(
            w2e.reshape(DC, P, M).transpose(1, 0, 2))           # [P,DC,M]
        in_maps.append({
            "xTs": np.ascontiguousarray(xT[:, e * TSH:(e + 1) * TSH]),
            "xb": xb, "wg": wgf,
            "w1p": w1pk, "w2p": w2pk,
            "b1v": np.ascontiguousarray(b1[e]).astype(np.float32),
            "b2b": np.tile(np.asarray(b2[e], dtype=np.float32), (P, 1)),
            "ecol": np.full((P, 1), float(e), dtype=np.float32),
            "toksf": toksf, "triu": triu, "identf": identf, "identb": identb,
            "pfill": pfill,
        })
    return in_maps


def kernel(x, wg, w1, b1, w2, b2, _trace=False):
    if "nc" not in _CACHE:
        _CACHE["nc"] = _build_nc()
    nc = _CACHE["nc"]
    in_maps = _prep_inputs(
        np.asarray(x), np.asarray(wg), np.asarray(w1),
        np.asarray(b1), np.asarray(w2), np.asarray(b2))
    res = run_bass_kernel_spmd(nc, in_maps, list(range(E)), trace=_trace)
    _CACHE["last_results"] = res
    full = np.zeros((T, M), dtype=np.float32)
    for e in range(E):
        full += res.results[e]["out"]
    return full.reshape(B, S, M)


# revision 15
# speedup vs baseline: 1.2597x; 1.0094x over previous
"""MoE top-1 routing kernel for 8 TRN2 NeuronCores (expert parallelism).

Self-contained: takes full inputs, shards experts across 8 cores, returns the
full output (host sums the 8 disjoint per-expert partials).

v3 design:
- Gating token-sharded: core j computes fp32 logits for its 1024 tokens with
  batched whole-shard DVE routing (argmax via masked reduce-min, gate =
  1/sum(exp(l-max))), then one AllGather shares [T, 2] (expert_id, gate).
- Positions: one triangular matmul + one tensor_tensor_scan; token->slot
  inverted via 64 row-scatters of (token_id, gate) pairs striped over 4
  BIG-prefilled DRAM arrays (4 independent completion chains), merged with
  selects on load. OOB slots silently dropped => capacity drop for free.
- FFN in bf16, w2 resident, w1 streamed twice over two 640-slot halves,
  fused bias+ReLU on ACT, gate-scaled rows scattered into pre-zeroed output.
"""
import numpy as np
import ml_dtypes
from contextlib import ExitStack

import concourse.bass as bass
import concourse.tile as tile
from concourse import bacc, mybir
from concourse.bass_utils import run_bass_kernel_spmd

dt = mybir.dt

B, S, M, E, DFF = 4, 2048, 1024, 8, 4096
T = B * S                  # 8192 tokens
C = int(1.25 * T / E)      # 1280 capacity
P = 128
NT = T // P                # 64 token tiles
MC = M // P                # 8 m chunks
DC = DFF // P              # 32 dff chunks
SCN = C // P               # 10 slot chunks
HALF = C // 2              # 640 slots per half
TSH = T // E               # 1024 tokens per shard
LT = TSH // P              # 8 local tiles
NIG = 4                    # scatter stripe count
BIG = 1.0e9
BIGI = 2 ** 30

_CACHE = {}


def _build_nc(stage=5):
    nc = bacc.Bacc("TRN2", target_bir_lowering=False, debug=False)

    # ---- I/O ----
    xTs = nc.dram_tensor("xTs", [M, TSH], dt.float32, kind="ExternalInput")
    xb = nc.dram_tensor("xb", [T, M], dt.bfloat16, kind="ExternalInput")
    wg = nc.dram_tensor("wg", [M, E], dt.float32, kind="ExternalInput")
    w1p = nc.dram_tensor("w1p", [DC, P, MC, P], dt.bfloat16, kind="ExternalInput")
    w2p = nc.dram_tensor("w2p", [P, DC, M], dt.bfloat16, kind="ExternalInput")
    b1v = nc.dram_tensor("b1v", [DFF], dt.float32, kind="ExternalInput")
    b2b = nc.dram_tensor("b2b", [P, M], dt.float32, kind="ExternalInput")
    ecol = nc.dram_tensor("ecol", [P, 1], dt.float32, kind="ExternalInput")
    eiota = nc.dram_tensor("eiota", [P, LT, E], dt.float32, kind="ExternalInput")
    toksf = nc.dram_tensor("toksf", [P, NT], dt.float32, kind="ExternalInput")
    triu = nc.dram_tensor("triu", [P, P], dt.float32, kind="ExternalInput")
    identf = nc.dram_tensor("identf", [P, P], dt.float32, kind="ExternalInput")
    identb = nc.dram_tensor("identb", [P, P], dt.bfloat16, kind="ExternalInput")
    pfill = nc.dram_tensor("pfill", [P, SCN, 2], dt.float32, kind="ExternalInput")
    outd = nc.dram_tensor("out", [T, M], dt.float32, kind="ExternalOutput")

    # ---- internal DRAM ----
    eg_loc = nc.dram_tensor("eg_loc", [TSH, 2], dt.float32)
    eg_all = nc.dram_tensor("eg_all", [T, 2], dt.float32, addr_space="Shared")
    igds = [nc.dram_tensor(f"igd{k}", [C, 2], dt.float32) for k in range(NIG)]

    with tile.TileContext(nc) as tc, ExitStack() as ctx:
        sb = ctx.enter_context(tc.tile_pool(name="sb", bufs=1))
        sbx = ctx.enter_context(tc.tile_pool(name="sbx", bufs=9))   # x stream
        sbw1 = ctx.enter_context(tc.tile_pool(name="sbw1", bufs=3))  # w1 stream
        sbg = ctx.enter_context(tc.tile_pool(name="sbg", bufs=3))   # gather tiles
        sbst = ctx.enter_context(tc.tile_pool(name="sbst", bufs=3))  # staging
        sbr = ctx.enter_context(tc.tile_pool(name="sbr", bufs=4))   # routing small

        # ---------- persistent tiles (small consts on sync queue) ----------
        wgt = sb.tile([P, MC * E], dt.float32)       # gate weights (mc, e)
        nc.sync.dma_start(wgt[:], wg[:].rearrange("(mc p) e -> p mc e", p=P))
        b1t = sb.tile([P, DC], dt.float32)           # b1 per-partition cols
        nc.sync.dma_start(b1t[:], b1v[:].rearrange("(d p) -> p d", p=P))
        ect = sb.tile([P, 1], dt.float32)
        nc.sync.dma_start(ect[:], ecol[:])
        eit = sb.tile([P, LT * E], dt.float32)
        nc.sync.dma_start(eit[:], eiota[:])
        tokf = sb.tile([P, NT], dt.float32)
        nc.sync.dma_start(tokf[:], toksf[:])
        trit = sb.tile([P, P], dt.float32)
        nc.sync.dma_start(trit[:], triu[:])
        idf = sb.tile([P, P], dt.float32)
        nc.sync.dma_start(idf[:], identf[:])
        idb = sb.tile([P, P], dt.bfloat16)
        nc.sync.dma_start(idb[:], identb[:])
        pft = sb.tile([P, SCN * 2], dt.float32)
        nc.sync.dma_start(pft[:], pfill[:])
        ones1 = sb.tile([1, P], dt.float32)
        nc.gpsimd.memset(ones1[:], 1.0)
        onescol = sb.tile([P, 1], dt.float32)
        nc.gpsimd.memset(onescol[:], 1.0)
        bigcol = sb.tile([P, NT], dt.float32)
        nc.gpsimd.memset(bigcol[:], BIG)
        nines = sb.tile([P, LT * E], dt.float32)
        nc.gpsimd.memset(nines[:], 9.0)
        # big tensors on the scalar (ACT) HWDGE queue so they don't block x
        b2t = sb.tile([P, M], dt.float32)
        nc.scalar.dma_start(b2t[:], b2b[:])
        w2t = sb.tile([P, DC * M], dt.bfloat16)      # resident w2 (d, m)
        if stage >= 5:
            for q in range(4):
                nc.scalar.dma_start(
                    w2t[:, q * 8 * M:(q + 1) * 8 * M],
                    w2p[:, q * 8:(q + 1) * 8, :])

        # prefill idx/gate stripes: BIG ids (skipped everywhere), zero gates
        for k in range(NIG):
            nc.sync.dma_start(
                igds[k][:].rearrange("(c p) two -> p c two", p=P), pft[:])

        eg_stk = sb.tile([P, LT * 2], dt.float32)    # local (eidx, gate) cols
        idx_t = sb.tile([P, SCN], dt.int32)
        gate_f = sb.tile([P, SCN], dt.float32)

        # ---------- phase A: sharded gating, batched routing ----------
        lg_stk = sb.tile([P, LT * E], dt.float32)
        with tc.tile_pool(name="psg", bufs=4, space="PSUM") as psg:
            xts = []
            for k in range(MC):
                xt = sbx.tile([P, TSH], dt.float32, tag="xt")
                nc.sync.dma_start(xt[:], xTs[k * P:(k + 1) * P, :])
                xts.append(xt)
            for ti in range(LT):
                pg = psg.tile([P, E], dt.float32, tag="pg")
                for k in range(MC):
                    nc.tensor.matmul(
                        pg[:], lhsT=xts[k][:, ti * P:(ti + 1) * P],
                        rhs=wgt[:, k * E:(k + 1) * E],
                        start=(k == 0), stop=(k == MC - 1))
                nc.vector.tensor_copy(lg_stk[:, ti * E:(ti + 1) * E], pg[:])
        lg3 = lg_stk[:].rearrange("p (ti e) -> p ti e", e=E)
        mx_stk = sb.tile([P, LT], dt.float32)
        nc.vector.tensor_reduce(
            out=mx_stk[:], in_=lg3, axis=mybir.AxisListType.X,
            op=mybir.AluOpType.max)
        mxb = mx_stk[:].rearrange("p (ti one) -> p ti one", one=1).to_broadcast([P, LT, E])
        ls = sb.tile([P, LT * E], dt.float32)
        nc.vector.tensor_tensor(
            out=ls[:].rearrange("p (ti e) -> p ti e", e=E), in0=lg3, in1=mxb,
            op=mybir.AluOpType.subtract)
        ex = sb.tile([P, LT * E], dt.float32)
        nc.scalar.activation(
            ex[:], ls[:], mybir.ActivationFunctionType.Exp)
        s_stk = sb.tile([P, LT], dt.float32)
        nc.vector.tensor_reduce(
            out=s_stk[:], in_=ex[:].rearrange("p (ti e) -> p ti e", e=E),
            axis=mybir.AxisListType.X, op=mybir.AluOpType.add)
        # gate into interleaved eg_stk col 2ti+1
        nc.vector.reciprocal(
            eg_stk[:].rearrange("p (ti two) -> p ti two", two=2)[:, :, 1:2],
            s_stk[:].rearrange("p (ti one) -> p ti one", one=1))
        # argmax with first-index tie-break: min over (onehot ? e : 9)
        oh = sb.tile([P, LT * E], dt.uint8)
        nc.vector.tensor_tensor(
            out=oh[:].rearrange("p (ti e) -> p ti e", e=E), in0=lg3, in1=mxb,
            op=mybir.AluOpType.is_equal)
        msk = sb.tile([P, LT * E], dt.float32)
        nc.vector.select(msk[:], oh[:], eit[:], nines[:])
        nc.vector.tensor_reduce(
            out=eg_stk[:].rearrange("p (ti two) -> p ti two", two=2)[:, :, 0:1],
            in_=msk[:].rearrange("p (ti e) -> p ti e", e=E),
            axis=mybir.AxisListType.X, op=mybir.AluOpType.min)

        # share routing metadata with all cores
        nc.sync.dma_start(
            eg_loc[:].rearrange("(t p) two -> p t two", p=P), eg_stk[:])
        nc.gpsimd.collective_compute(
            "AllGather", mybir.AluOpType.bypass,
            ins=[eg_loc[:]], outs=[eg_all[:]],
            replica_groups=[list(range(E))])

        if stage < 2:
            nc.compile()
            return nc

        # ---------- phase B: batched positions + token->slot inversion -----
        eidx_stk = sb.tile([P, NT], dt.float32)
        gate_stk = sb.tile([P, NT], dt.float32)
        nc.sync.dma_start(
            eidx_stk[:],
            eg_all[:].rearrange("(t p) two -> p t two", p=P)[:, :, 0:1])
        nc.sync.dma_start(
            gate_stk[:],
            eg_all[:].rearrange("(t p) two -> p t two", p=P)[:, :, 1:2])
        mine_stk = sb.tile([P, NT], dt.float32)
        nc.vector.tensor_scalar(
            out=mine_stk[:], in0=eidx_stk[:], scalar1=ect[:, 0:1], scalar2=None,
            op0=mybir.AluOpType.is_equal)

        with tc.tile_pool(name="ppb", bufs=1, space="PSUM") as ppb:
            # tile sums -> [1, NT] row via matmul + transpose
            pts = ppb.tile([64, 1], dt.float32, tag="pts")
            nc.tensor.matmul(pts[:], lhsT=mine_stk[:], rhs=onescol[:],
                             start=True, stop=True)
            tcol = sb.tile([64, 1], dt.float32)
            nc.vector.tensor_copy(tcol[:], pts[:])
            ptr = ppb.tile([1, 64], dt.float32, tag="ptr")
            nc.tensor.transpose(out=ptr[:], in_=tcol[:], identity=idf[:64, :64])
            tsums = sb.tile([1, NT], dt.float32)
            nc.vector.tensor_copy(tsums[:], ptr[:])
            zrow = sb.tile([1, NT], dt.float32)
            nc.gpsimd.memset(zrow[:], 0.0)
            incl = sb.tile([1, NT], dt.float32)
            nc.vector.tensor_tensor_scan(
                out=incl[:], data0=tsums[:], data1=zrow[:], initial=0.0,
                op0=mybir.AluOpType.add, op1=mybir.AluOpType.add)
            offs = sb.tile([1, NT], dt.float32)
            nc.vector.tensor_tensor(
                out=offs[:], in0=incl[:], in1=tsums[:],
                op=mybir.AluOpType.subtract)
            # batched positions for all 64 tiles
            pall = ppb.tile([P, NT], dt.float32, tag="pall")
            nc.tensor.matmul(pall[:], lhsT=trit[:], rhs=mine_stk[:],
                             start=True, stop=False)
            nc.tensor.matmul(pall[:], lhsT=ones1[:], rhs=offs[:],
                             start=False, stop=True)
            slotm1 = sb.tile([P, NT], dt.float32)
            nc.vector.tensor_scalar_add(slotm1[:], pall[:], -1.0)
        mu8 = sb.tile([P, NT], dt.uint8)
        nc.vector.tensor_scalar(
            out=mu8[:], in0=mine_stk[:], scalar1=0.5, scalar2=None,
            op0=mybir.AluOpType.is_gt)
        slotf = sb.tile([P, NT], dt.float32)
        nc.vector.select(slotf[:], mu8[:], slotm1[:], bigcol[:])
        sloti = sb.tile([P, NT], dt.int32)
        nc.vector.tensor_copy(sloti[:], slotf[:])
        # (token_id, gate) pairs; interleaved columns
        pairs = sb.tile([P, NT * 2], dt.float32)
        nc.vector.tensor_copy(
            pairs[:].rearrange("p (t two) -> p t two", two=2)[:, :, 0:1],
            tokf[:])
        nc.vector.tensor_copy(
            pairs[:].rearrange("p (t two) -> p t two", two=2)[:, :, 1:2],
            gate_stk[:])
        for t in range(NT):
            nc.gpsimd.indirect_dma_start(
                out=igds[t % NIG][:], out_offset=bass.IndirectOffsetOnAxis(
                    ap=sloti[:, t:t + 1], axis=0),
                in_=pairs[:, 2 * t:2 * t + 2], in_offset=None,
                bounds_check=C - 1, oob_is_err=False)
        # merge the stripes: each slot is filled in at most one stripe
        idxf = sb.tile([P, SCN], dt.float32)
        nc.sync.dma_start(
            idxf[:],
            igds[0][:].rearrange("(c p) two -> p c two", p=P)[:, :, 0:1])
        nc.sync.dma_start(
            gate_f[:],
            igds[0][:].rearrange("(c p) two -> p c two", p=P)[:, :, 1:2])
        for k in range(1, NIG):
            ikf = sbr.tile([P, SCN], dt.float32, tag="ikf")
            gkf = sbr.tile([P, SCN], dt.float32, tag="gkf")
            nc.sync.dma_start(
                ikf[:],
                igds[k][:].rearrange("(c p) two -> p c two", p=P)[:, :, 0:1])
            nc.sync.dma_start(
                gkf[:],
                igds[k][:].rearrange("(c p) two -> p c two", p=P)[:, :, 1:2])
            vk = sbr.tile([P, SCN], dt.uint8, tag="vk")
            nc.vector.tensor_scalar(
                out=vk[:], in0=ikf[:], scalar1=BIG * 0.5, scalar2=None,
                op0=mybir.AluOpType.is_lt)
            nc.vector.copy_predicated(idxf[:], vk[:], ikf[:])
            nc.vector.copy_predicated(gate_f[:], vk[:], gkf[:])
        nc.vector.tensor_copy(idx_t[:], idxf[:])

        if stage < 3:
            nc.compile()
            return nc

        # ---------- phases C/D/E per half ----------
        with (
            tc.tile_pool(name="pstr", bufs=2, space="PSUM") as pstr,
            tc.tile_pool(name="ps1", bufs=2, space="PSUM") as ps1,
            tc.tile_pool(name="ps2", bufs=2, space="PSUM") as ps2,
        ):
            for h in range(2):
                dispT = sb.tile([P, MC * HALF], dt.bfloat16, tag="dispT")
                hT = sb.tile([P, DC * HALF], dt.bfloat16, tag="hT")
                # dispatch: gather + transpose
                for s5 in range(5):
                    sc = h * 5 + s5
                    gx = sbg.tile([P, M], dt.bfloat16, tag="gx")
                    nc.gpsimd.memset(gx[:], 0.0)
                    nc.gpsimd.indirect_dma_start(
                        out=gx[:], out_offset=None, in_=xb[:],
                        in_offset=bass.IndirectOffsetOnAxis(
                            ap=idx_t[:, sc:sc + 1], axis=0),
                        bounds_check=T - 1, oob_is_err=False)
                    for mm in range(MC):
                        ptg = pstr.tile([P, P], dt.bfloat16, tag="ptg")
                        nc.tensor.transpose(
                            out=ptg[:], in_=gx[:, mm * P:(mm + 1) * P],
                            identity=idb[:])
                        nc.vector.tensor_copy(
                            dispT[:, mm * HALF + s5 * P:
                                  mm * HALF + (s5 + 1) * P],
                            ptg[:])
                # FFN1
                if stage >= 4:
                    for d in range(DC):
                        w1t = sbw1.tile([P, M], dt.bfloat16, tag="w1t")
                        nc.sync.dma_start(w1t[:], w1p[d])
                        pA = ps1.tile([P, 512], dt.float32, tag="pA")
                        pB = ps1.tile([P, P], dt.float32, tag="pB")
                        for mc in range(MC):
                            lhs = w1t[:, mc * P:(mc + 1) * P]
                            nc.tensor.matmul(
                                pA[:], lhsT=lhs,
                                rhs=dispT[:, mc * HALF:mc * HALF + 512],
                                start=(mc == 0), stop=(mc == MC - 1))
                            nc.tensor.matmul(
                                pB[:], lhsT=lhs,
                                rhs=dispT[:, mc * HALF + 512:(mc + 1) * HALF],
                                start=(mc == 0), stop=(mc == MC - 1))
                        nc.scalar.activation(
                            hT[:, d * HALF:d * HALF + 512], pA[:],
                            mybir.ActivationFunctionType.Relu,
                            bias=b1t[:, d:d + 1], scale=1.0)
                        nc.scalar.activation(
                            hT[:, d * HALF + 512:(d + 1) * HALF], pB[:],
                            mybir.ActivationFunctionType.Relu,
                            bias=b1t[:, d:d + 1], scale=1.0)
                # FFN2 + combine + scatter
                if stage >= 5:
                    for s5 in range(5):
                        sc = h * 5 + s5
                        st = sbst.tile([P, M], dt.float32, tag="st")
                        for mm in range(2):
                            po = ps2.tile([P, 512], dt.float32, tag="po")
                            for d in range(DC):
                                nc.tensor.matmul(
                                    po[:],
                                    lhsT=hT[:, d * HALF + s5 * P:
                                            d * HALF + (s5 + 1) * P],
                                    rhs=w2t[:, d * M + mm * 512:
                                            d * M + (mm + 1) * 512],
                                    start=(d == 0), stop=(d == DC - 1))
                            nc.vector.tensor_tensor(
                                out=st[:, mm * 512:(mm + 1) * 512], in0=po[:],
                                in1=b2t[:, mm * 512:(mm + 1) * 512],
                                op=mybir.AluOpType.add)
                        nc.vector.tensor_scalar_mul(
                            st[:], st[:], gate_f[:, sc:sc + 1])
                        nc.gpsimd.indirect_dma_start(
                            out=outd[:], out_offset=bass.IndirectOffsetOnAxis(
                                ap=idx_t[:, sc:sc + 1], axis=0),
                            in_=st[:], in_offset=None,
                            bounds_check=T - 1, oob_is_err=False)

    nc.compile()
    return nc


def _prep_inputs(x, wg, w1, b1, w2, b2):
    bf16 = ml_dtypes.bfloat16
    tokens = np.ascontiguousarray(x.reshape(T, M)).astype(np.float32)
    xT = np.ascontiguousarray(tokens.T)
    xb = tokens.astype(bf16)
    wgf = np.ascontiguousarray(wg.astype(np.float32))
    toksf = np.arange(T, dtype=np.float32).reshape(NT, P).T.copy()
    eiota = np.broadcast_to(
        np.arange(E, dtype=np.float32), (P, LT, E)).copy()
    triu = np.triu(np.ones((P, P), dtype=np.float32))
    identf = np.eye(P, dtype=np.float32)
    identb = np.eye(P).astype(bf16)
    pfill = np.zeros((P, SCN, 2), dtype=np.float32)
    pfill[:, :, 0] = BIG
    in_maps = []
    for e in range(E):
        w1e = np.ascontiguousarray(w1[e]).astype(bf16)          # [M, DFF]
        w1pk = np.ascontiguousarray(
            w1e.reshape(MC, P, DC, P).transpose(2, 1, 0, 3))    # [DC,P,MC,P]
        w2e = np.ascontiguousarray(w2[e]).astype(bf16)          # [DFF, M]
        w2pk = np.ascontiguousarray(
            w2e.reshape(DC, P, M).transpose(1, 0, 2))           # [P,DC,M]
        in_maps.append({
            "xTs": np.ascontiguousarray(xT[:, e * TSH:(e + 1) * TSH]),
            "xb": xb, "wg": wgf,
            "w1p": w1pk, "w2p": w2pk,
            "b1v": np.ascontiguousarray(b1[e]).astype(np.float32),
            "b2b": np.tile(np.asarray(b2[e], dtype=np.float32), (P, 1)),
            "ecol": np.full((P, 1), float(e), dtype=np.float32),
            "eiota": eiota, "toksf": toksf, "triu": triu,
            "identf": identf, "identb": identb, "pfill": pfill,
        })
    return in_maps


def kernel(x, wg, w1, b1, w2, b2, _trace=False):
    if "nc" not in _CACHE:
        _CACHE["nc"] = _build_nc()
    nc = _CACHE["nc"]
    in_maps = _prep_inputs(
        np.asarray(x), np.asarray(wg), np.asarray(w1),
        np.asarray(b1), np.asarray(w2), np.asarray(b2))
    res = run_bass_kernel_spmd(nc, in_maps, list(range(E)), trace=_trace)
    _CACHE["last_results"] = res
    full = np.zeros((T, M), dtype=np.float32)
    for e in range(E):
        full += res.results[e]["out"]
    return full.reshape(B, S, M)


# revision 17
# speedup vs baseline: 1.2789x; 1.0152x over previous
"""MoE top-1 routing kernel for 8 TRN2 NeuronCores (expert parallelism).

Self-contained: takes full inputs, shards experts across 8 cores, returns the
full output (host sums the 8 disjoint per-expert partials).

v3 design:
- Gating token-sharded: core j computes fp32 logits for its 1024 tokens with
  batched whole-shard DVE routing (argmax via masked reduce-min, gate =
  1/sum(exp(l-max))), then one AllGather shares [T, 2] (expert_id, gate).
- Positions: one triangular matmul + one tensor_tensor_scan; token->slot
  inverted via 64 row-scatters of (token_id, gate) pairs striped over 4
  BIG-prefilled DRAM arrays (4 independent completion chains), merged with
  selects on load. OOB slots silently dropped => capacity drop for free.
- FFN in bf16, w2 resident, w1 streamed twice over two 640-slot halves,
  fused bias+ReLU on ACT, gate-scaled rows scattered into pre-zeroed output.
"""
import numpy as np
import ml_dtypes
from contextlib import ExitStack

import concourse.bass as bass
import concourse.tile as tile
from concourse import bacc, mybir
from concourse.bass_utils import run_bass_kernel_spmd

dt = mybir.dt

B, S, M, E, DFF = 4, 2048, 1024, 8, 4096
T = B * S                  # 8192 tokens
C = int(1.25 * T / E)      # 1280 capacity
P = 128
NT = T // P                # 64 token tiles
MC = M // P                # 8 m chunks
DC = DFF // P              # 32 dff chunks
SCN = C // P               # 10 slot chunks
HALF = C // 2              # 640 slots per half
TSH = T // E               # 1024 tokens per shard
LT = TSH // P              # 8 local tiles
NIG = 8                    # scatter stripe count
BIG = 1.0e9
BIGI = 2 ** 30

_CACHE = {}


def _build_nc(stage=5):
    nc = bacc.Bacc("TRN2", target_bir_lowering=False, debug=False)

    # ---- I/O ----
    xTs = nc.dram_tensor("xTs", [M, TSH], dt.float32, kind="ExternalInput")
    xb = nc.dram_tensor("xb", [T, M], dt.bfloat16, kind="ExternalInput")
    wg = nc.dram_tensor("wg", [M, E], dt.float32, kind="ExternalInput")
    w1p = nc.dram_tensor("w1p", [DC, P, MC, P], dt.bfloat16, kind="ExternalInput")
    w2p = nc.dram_tensor("w2p", [P, DC, M], dt.bfloat16, kind="ExternalInput")
    b1v = nc.dram_tensor("b1v", [DFF], dt.float32, kind="ExternalInput")
    b2b = nc.dram_tensor("b2b", [P, M], dt.float32, kind="ExternalInput")
    ecol = nc.dram_tensor("ecol", [P, 1], dt.float32, kind="ExternalInput")
    eiota = nc.dram_tensor("eiota", [P, LT, E], dt.float32, kind="ExternalInput")
    toksf = nc.dram_tensor("toksf", [P, NT], dt.float32, kind="ExternalInput")
    triu = nc.dram_tensor("triu", [P, P], dt.float32, kind="ExternalInput")
    identf = nc.dram_tensor("identf", [P, P], dt.float32, kind="ExternalInput")
    identb = nc.dram_tensor("identb", [P, P], dt.bfloat16, kind="ExternalInput")
    pfill = nc.dram_tensor("pfill", [P, SCN, 2], dt.float32, kind="ExternalInput")
    outd = nc.dram_tensor("out", [T, M], dt.float32, kind="ExternalOutput")

    # ---- internal DRAM ----
    eg_loc = nc.dram_tensor("eg_loc", [TSH, 2], dt.float32)
    eg_all = nc.dram_tensor("eg_all", [T, 2], dt.float32, addr_space="Shared")
    igds = [nc.dram_tensor(f"igd{k}", [C, 2], dt.float32) for k in range(NIG)]

    with tile.TileContext(nc) as tc, ExitStack() as ctx:
        sb = ctx.enter_context(tc.tile_pool(name="sb", bufs=1))
        sbx = ctx.enter_context(tc.tile_pool(name="sbx", bufs=9))   # x stream
        sbw1 = ctx.enter_context(tc.tile_pool(name="sbw1", bufs=3))  # w1 stream
        sbg = ctx.enter_context(tc.tile_pool(name="sbg", bufs=3))   # gather tiles
        sbst = ctx.enter_context(tc.tile_pool(name="sbst", bufs=3))  # staging
        sbr = ctx.enter_context(tc.tile_pool(name="sbr", bufs=4))   # routing small

        # ---------- persistent tiles (small consts on sync queue) ----------
        wgt = sb.tile([P, MC * E], dt.float32)       # gate weights (mc, e)
        nc.sync.dma_start(wgt[:], wg[:].rearrange("(mc p) e -> p mc e", p=P))
        b1t = sb.tile([P, DC], dt.float32)           # b1 per-partition cols
        nc.sync.dma_start(b1t[:], b1v[:].rearrange("(d p) -> p d", p=P))
        ect = sb.tile([P, 1], dt.float32)
        nc.sync.dma_start(ect[:], ecol[:])
        eit = sb.tile([P, LT * E], dt.float32)
        nc.sync.dma_start(eit[:], eiota[:])
        tokf = sb.tile([P, NT], dt.float32)
        nc.sync.dma_start(tokf[:], toksf[:])
        trit = sb.tile([P, P], dt.float32)
        nc.sync.dma_start(trit[:], triu[:])
        idf = sb.tile([P, P], dt.float32)
        nc.sync.dma_start(idf[:], identf[:])
        idb = sb.tile([P, P], dt.bfloat16)
        nc.sync.dma_start(idb[:], identb[:])
        pft = sb.tile([P, SCN * 2], dt.float32)
        nc.sync.dma_start(pft[:], pfill[:])
        ones1 = sb.tile([1, P], dt.float32)
        nc.gpsimd.memset(ones1[:], 1.0)
        onescol = sb.tile([P, 1], dt.float32)
        nc.gpsimd.memset(onescol[:], 1.0)
        bigcol = sb.tile([P, NT], dt.float32)
        nc.gpsimd.memset(bigcol[:], BIG)
        nines = sb.tile([P, LT * E], dt.float32)
        nc.gpsimd.memset(nines[:], 9.0)
        # big tensors on the scalar (ACT) HWDGE queue so they don't block x
        b2t = sb.tile([P, M], dt.float32)
        nc.scalar.dma_start(b2t[:], b2b[:])
        w2t = sb.tile([P, DC * M], dt.bfloat16)      # resident w2 (d, m)
        if stage >= 5:
            for q in range(4):
                nc.scalar.dma_start(
                    w2t[:, q * 8 * M:(q + 1) * 8 * M],
                    w2p[:, q * 8:(q + 1) * 8, :])

        # prefill idx/gate stripes: BIG ids (skipped everywhere), zero gates
        for k in range(NIG):
            nc.sync.dma_start(
                igds[k][:].rearrange("(c p) two -> p c two", p=P), pft[:])

        eg_stk = sb.tile([P, LT * 2], dt.float32)    # local (eidx, gate) cols
        idx_t = sb.tile([P, SCN], dt.int32)
        gate_f = sb.tile([P, SCN], dt.float32)

        # ---------- phase A: sharded gating, batched routing ----------
        lg_stk = sb.tile([P, LT * E], dt.float32)
        with tc.tile_pool(name="psg", bufs=8, space="PSUM") as psg:
            xts = []
            for k in range(MC):
                xt = sbx.tile([P, TSH], dt.float32, tag="xt")
                nc.sync.dma_start(xt[:], xTs[k * P:(k + 1) * P, :])
                xts.append(xt)
            for ti in range(LT):
                pg = psg.tile([P, E], dt.float32, tag="pg")
                for k in range(MC):
                    nc.tensor.matmul(
                        pg[:], lhsT=xts[k][:, ti * P:(ti + 1) * P],
                        rhs=wgt[:, k * E:(k + 1) * E],
                        start=(k == 0), stop=(k == MC - 1))
                nc.vector.tensor_copy(lg_stk[:, ti * E:(ti + 1) * E], pg[:])
        lg3 = lg_stk[:].rearrange("p (ti e) -> p ti e", e=E)
        mx_stk = sb.tile([P, LT], dt.float32)
        nc.vector.tensor_reduce(
            out=mx_stk[:], in_=lg3, axis=mybir.AxisListType.X,
            op=mybir.AluOpType.max)
        mxb = mx_stk[:].rearrange("p (ti one) -> p ti one", one=1).to_broadcast([P, LT, E])
        ls = sb.tile([P, LT * E], dt.float32)
        nc.vector.tensor_tensor(
            out=ls[:].rearrange("p (ti e) -> p ti e", e=E), in0=lg3, in1=mxb,
            op=mybir.AluOpType.subtract)
        ex = sb.tile([P, LT * E], dt.float32)
        nc.scalar.activation(
            ex[:], ls[:], mybir.ActivationFunctionType.Exp)
        s_stk = sb.tile([P, LT], dt.float32)
        nc.vector.tensor_reduce(
            out=s_stk[:], in_=ex[:].rearrange("p (ti e) -> p ti e", e=E),
            axis=mybir.AxisListType.X, op=mybir.AluOpType.add)
        # gate into interleaved eg_stk col 2ti+1
        nc.vector.reciprocal(
            eg_stk[:].rearrange("p (ti two) -> p ti two", two=2)[:, :, 1:2],
            s_stk[:].rearrange("p (ti one) -> p ti one", one=1))
        # argmax with first-index tie-break: min over (onehot ? e : 9)
        oh = sb.tile([P, LT * E], dt.uint8)
        nc.vector.tensor_tensor(
            out=oh[:].rearrange("p (ti e) -> p ti e", e=E), in0=lg3, in1=mxb,
            op=mybir.AluOpType.is_equal)
        msk = sb.tile([P, LT * E], dt.float32)
        nc.vector.select(msk[:], oh[:], eit[:], nines[:])
        nc.vector.tensor_reduce(
            out=eg_stk[:].rearrange("p (ti two) -> p ti two", two=2)[:, :, 0:1],
            in_=msk[:].rearrange("p (ti e) -> p ti e", e=E),
            axis=mybir.AxisListType.X, op=mybir.AluOpType.min)

        # share routing metadata with all cores
        nc.sync.dma_start(
            eg_loc[:].rearrange("(t p) two -> p t two", p=P), eg_stk[:])
        nc.gpsimd.collective_compute(
            "AllGather", mybir.AluOpType.bypass,
            ins=[eg_loc[:]], outs=[eg_all[:]],
            replica_groups=[list(range(E))])

        if stage < 2:
            nc.compile()
            return nc

        # ---------- phase B: batched positions + token->slot inversion -----
        eidx_stk = sb.tile([P, NT], dt.float32)
        gate_stk = sb.tile([P, NT], dt.float32)
        nc.sync.dma_start(
            eidx_stk[:],
            eg_all[:].rearrange("(t p) two -> p t two", p=P)[:, :, 0:1])
        nc.sync.dma_start(
            gate_stk[:],
            eg_all[:].rearrange("(t p) two -> p t two", p=P)[:, :, 1:2])
        mine_stk = sb.tile([P, NT], dt.float32)
        nc.vector.tensor_scalar(
            out=mine_stk[:], in0=eidx_stk[:], scalar1=ect[:, 0:1], scalar2=None,
            op0=mybir.AluOpType.is_equal)

        with tc.tile_pool(name="ppb", bufs=1, space="PSUM") as ppb:
            # tile sums -> [1, NT] row via matmul + transpose
            pts = ppb.tile([64, 1], dt.float32, tag="pts")
            nc.tensor.matmul(pts[:], lhsT=mine_stk[:], rhs=onescol[:],
                             start=True, stop=True)
            tcol = sb.tile([64, 1], dt.float32)
            nc.vector.tensor_copy(tcol[:], pts[:])
            ptr = ppb.tile([1, 64], dt.float32, tag="ptr")
            nc.tensor.transpose(out=ptr[:], in_=tcol[:], identity=idf[:64, :64])
            tsums = sb.tile([1, NT], dt.float32)
            nc.vector.tensor_copy(tsums[:], ptr[:])
            zrow = sb.tile([1, NT], dt.float32)
            nc.gpsimd.memset(zrow[:], 0.0)
            incl = sb.tile([1, NT], dt.float32)
            nc.vector.tensor_tensor_scan(
                out=incl[:], data0=tsums[:], data1=zrow[:], initial=0.0,
                op0=mybir.AluOpType.add, op1=mybir.AluOpType.add)
            offs = sb.tile([1, NT], dt.float32)
            nc.vector.tensor_tensor(
                out=offs[:], in0=incl[:], in1=tsums[:],
                op=mybir.AluOpType.subtract)
            # batched positions for all 64 tiles
            pall = ppb.tile([P, NT], dt.float32, tag="pall")
            nc.tensor.matmul(pall[:], lhsT=trit[:], rhs=mine_stk[:],
                             start=True, stop=False)
            nc.tensor.matmul(pall[:], lhsT=ones1[:], rhs=offs[:],
                             start=False, stop=True)
            slotm1 = sb.tile([P, NT], dt.float32)
            nc.vector.tensor_scalar_add(slotm1[:], pall[:], -1.0)
        mu8 = sb.tile([P, NT], dt.uint8)
        nc.vector.tensor_scalar(
            out=mu8[:], in0=mine_stk[:], scalar1=0.5, scalar2=None,
            op0=mybir.AluOpType.is_gt)
        slotf = sb.tile([P, NT], dt.float32)
        nc.vector.select(slotf[:], mu8[:], slotm1[:], bigcol[:])
        sloti = sb.tile([P, NT], dt.int32)
        nc.vector.tensor_copy(sloti[:], slotf[:])
        # (token_id, gate) pairs; interleaved columns
        pairs = sb.tile([P, NT * 2], dt.float32)
        nc.vector.tensor_copy(
            pairs[:].rearrange("p (t two) -> p t two", two=2)[:, :, 0:1],
            tokf[:])
        nc.vector.tensor_copy(
            pairs[:].rearrange("p (t two) -> p t two", two=2)[:, :, 1:2],
            gate_stk[:])
        for t in range(NT):
            nc.gpsimd.indirect_dma_start(
                out=igds[t % NIG][:], out_offset=bass.IndirectOffsetOnAxis(
                    ap=sloti[:, t:t + 1], axis=0),
                in_=pairs[:, 2 * t:2 * t + 2], in_offset=None,
                bounds_check=C - 1, oob_is_err=False)
        # merge the stripes: each slot is filled in at most one stripe
        idxf = sb.tile([P, SCN], dt.float32)
        nc.sync.dma_start(
            idxf[:],
            igds[0][:].rearrange("(c p) two -> p c two", p=P)[:, :, 0:1])
        nc.sync.dma_start(
            gate_f[:],
            igds[0][:].rearrange("(c p) two -> p c two", p=P)[:, :, 1:2])
        for k in range(1, NIG):
            ikf = sbr.tile([P, SCN], dt.float32, tag="ikf")
            gkf = sbr.tile([P, SCN], dt.float32, tag="gkf")
            nc.sync.dma_start(
                ikf[:],
                igds[k][:].rearrange("(c p) two -> p c two", p=P)[:, :, 0:1])
            nc.sync.dma_start(
                gkf[:],
                igds[k][:].rearrange("(c p) two -> p c two", p=P)[:, :, 1:2])
            vk = sbr.tile([P, SCN], dt.uint8, tag="vk")
            nc.vector.tensor_scalar(
                out=vk[:], in0=ikf[:], scalar1=BIG * 0.5, scalar2=None,
                op0=mybir.AluOpType.is_lt)
            nc.vector.copy_predicated(idxf[:], vk[:], ikf[:])
            nc.vector.copy_predicated(gate_f[:], vk[:], gkf[:])
        nc.vector.tensor_copy(idx_t[:], idxf[:])

        if stage < 3:
            nc.compile()
            return nc

        # ---------- phases C/D/E per half ----------
        with (
            tc.tile_pool(name="pstr", bufs=2, space="PSUM") as pstr,
            tc.tile_pool(name="ps1", bufs=2, space="PSUM") as ps1,
            tc.tile_pool(name="ps2", bufs=2, space="PSUM") as ps2,
        ):
            for h in range(2):
                dispT = sb.tile([P, MC * HALF], dt.bfloat16, tag="dispT")
                hT = sb.tile([P, DC * HALF], dt.bfloat16, tag="hT")
                # dispatch: gather + transpose
                for s5 in range(5):
                    sc = h * 5 + s5
                    gx = sbg.tile([P, M], dt.bfloat16, tag="gx")
                    nc.gpsimd.memset(gx[:], 0.0)
                    nc.gpsimd.indirect_dma_start(
                        out=gx[:], out_offset=None, in_=xb[:],
                        in_offset=bass.IndirectOffsetOnAxis(
                            ap=idx_t[:, sc:sc + 1], axis=0),
                        bounds_check=T - 1, oob_is_err=False)
                    for mm in range(MC):
                        ptg = pstr.tile([P, P], dt.bfloat16, tag="ptg")
                        nc.tensor.transpose(
                            out=ptg[:], in_=gx[:, mm * P:(mm + 1) * P],
                            identity=idb[:])
                        nc.vector.tensor_copy(
                            dispT[:, mm * HALF + s5 * P:
                                  mm * HALF + (s5 + 1) * P],
                            ptg[:])
                # FFN1
                if stage >= 4:
                    for d in range(DC):
                        w1t = sbw1.tile([P, M], dt.bfloat16, tag="w1t")
                        nc.sync.dma_start(w1t[:], w1p[d])
                        pA = ps1.tile([P, 512], dt.float32, tag="pA")
                        pB = ps1.tile([P, P], dt.float32, tag="pB")
                        for mc in range(MC):
                            lhs = w1t[:, mc * P:(mc + 1) * P]
                            nc.tensor.matmul(
                                pA[:], lhsT=lhs,
                                rhs=dispT[:, mc * HALF:mc * HALF + 512],
                                start=(mc == 0), stop=(mc == MC - 1))
                            nc.tensor.matmul(
                                pB[:], lhsT=lhs,
                                rhs=dispT[:, mc * HALF + 512:(mc + 1) * HALF],
                                start=(mc == 0), stop=(mc == MC - 1))
                        nc.scalar.activation(
                            hT[:, d * HALF:d * HALF + 512], pA[:],
                            mybir.ActivationFunctionType.Relu,
                            bias=b1t[:, d:d + 1], scale=1.0)
                        nc.scalar.activation(
                            hT[:, d * HALF + 512:(d + 1) * HALF], pB[:],
                            mybir.ActivationFunctionType.Relu,
                            bias=b1t[:, d:d + 1], scale=1.0)
                # FFN2 + combine + scatter
                if stage >= 5:
                    for s5 in range(5):
                        sc = h * 5 + s5
                        st = sbst.tile([P, M], dt.float32, tag="st")
                        for mm in range(2):
                            po = ps2.tile([P, 512], dt.float32, tag="po")
                            for d in range(DC):
                                nc.tensor.matmul(
                                    po[:],
                                    lhsT=hT[:, d * HALF + s5 * P:
                                            d * HALF + (s5 + 1) * P],
                                    rhs=w2t[:, d * M + mm * 512:
                                            d * M + (mm + 1) * 512],
                                    start=(d == 0), stop=(d == DC - 1))
                            nc.vector.tensor_tensor(
                                out=st[:, mm * 512:(mm + 1) * 512], in0=po[:],
                                in1=b2t[:, mm * 512:(mm + 1) * 512],
                                op=mybir.AluOpType.add)
                        nc.vector.tensor_scalar_mul(
                            st[:], st[:], gate_f[:, sc:sc + 1])
                        nc.gpsimd.indirect_dma_start(
                            out=outd[:], out_offset=bass.IndirectOffsetOnAxis(
                                ap=idx_t[:, sc:sc + 1], axis=0),
                            in_=st[:], in_offset=None,
                            bounds_check=T - 1, oob_is_err=False)

    nc.compile()
    return nc


def _prep_inputs(x, wg, w1, b1, w2, b2):
    bf16 = ml_dtypes.bfloat16
    tokens = np.ascontiguousarray(x.reshape(T, M)).astype(np.float32)
    xT = np.ascontiguousarray(tokens.T)
    xb = tokens.astype(bf16)
    wgf = np.ascontiguousarray(wg.astype(np.float32))
    toksf = np.arange(T, dtype=np.float32).reshape(NT, P).T.copy()
    eiota = np.broadcast_to(
        np.arange(E, dtype=np.float32), (P, LT, E)).copy()
    triu = np.triu(np.ones((P, P), dtype=np.float32))
    identf = np.eye(P, dtype=np.float32)
    identb = np.eye(P).astype(bf16)
    pfill = np.zeros((P, SCN, 2), dtype=np.float32)
    pfill[:, :, 0] = BIG
    in_maps = []
    for e in range(E):
        w1e = np.ascontiguousarray(w1[e]).astype(bf16)          # [M, DFF]
        w1pk = np.ascontiguousarray(
            w1e.reshape(MC, P, DC, P).transpose(2, 1, 0, 3))    # [DC,P,MC,P]
        w2e = np.ascontiguousarray(w2[e]).astype(bf16)          # [DFF, M]
        w2pk = np.ascontiguousarray(
            w2e.reshape(DC, P, M).transpose(1, 0, 2))           # [P,DC,M]
        in_maps.append({
            "xTs": np.ascontiguousarray(xT[:, e * TSH:(e + 1) * TSH]),
            "xb": xb, "wg": wgf,
            "w1p": w1pk, "w2p": w2pk,
            "b1v": np.ascontiguousarray(b1[e]).astype(np.float32),
            "b2b": np.tile(np.asarray(b2[e], dtype=np.float32), (P, 1)),
            "ecol": np.full((P, 1), float(e), dtype=np.float32),
            "eiota": eiota, "toksf": toksf, "triu": triu,
            "identf": identf, "identb": identb, "pfill": pfill,
        })
    return in_maps


def kernel(x, wg, w1, b1, w2, b2, _trace=False):
    if "nc" not in _CACHE:
        _CACHE["nc"] = _build_nc()
    nc = _CACHE["nc"]
    in_maps = _prep_inputs(
        np.asarray(x), np.asarray(wg), np.asarray(w1),
        np.asarray(b1), np.asarray(w2), np.asarray(b2))
    res = run_bass_kernel_spmd(nc, in_maps, list(range(E)), trace=_trace)
    _CACHE["last_results"] = res
    full = np.zeros((T, M), dtype=np.float32)
    for e in range(E):
        full += res.results[e]["out"]
    return full.reshape(B, S, M)


# revision 18
# speedup vs baseline: 1.2972x; 1.0143x over previous
"""MoE top-1 routing kernel for 8 TRN2 NeuronCores (expert parallelism).

Self-contained: takes full inputs, shards experts across 8 cores, returns the
full output (host sums the 8 disjoint per-expert partials).

v3 design:
- Gating token-sharded: core j computes fp32 logits for its 1024 tokens with
  batched whole-shard DVE routing (argmax via masked reduce-min, gate =
  1/sum(exp(l-max))), then one AllGather shares [T, 2] (expert_id, gate).
- Positions: one triangular matmul + one tensor_tensor_scan; token->slot
  inverted via 64 row-scatters of (token_id, gate) pairs striped over 4
  BIG-prefilled DRAM arrays (4 independent completion chains), merged with
  selects on load. OOB slots silently dropped => capacity drop for free.
- FFN in bf16, w2 resident, w1 streamed twice over two 640-slot halves,
  fused bias+ReLU on ACT, gate-scaled rows scattered into pre-zeroed output.
"""
import numpy as np
import ml_dtypes
from contextlib import ExitStack

import concourse.bass as bass
import concourse.tile as tile
from concourse import bacc, mybir
from concourse.bass_utils import run_bass_kernel_spmd

dt = mybir.dt

B, S, M, E, DFF = 4, 2048, 1024, 8, 4096
T = B * S                  # 8192 tokens
C = int(1.25 * T / E)      # 1280 capacity
P = 128
NT = T // P                # 64 token tiles
MC = M // P                # 8 m chunks
DC = DFF // P              # 32 dff chunks
SCN = C // P               # 10 slot chunks
HALF = C // 2              # 640 slots per half
TSH = T // E               # 1024 tokens per shard
LT = TSH // P              # 8 local tiles
NIG = 8                    # scatter stripe count
BIG = 1.0e9
BIGI = 2 ** 30

_CACHE = {}


def _build_nc(stage=5):
    nc = bacc.Bacc("TRN2", target_bir_lowering=False, debug=False)

    # ---- I/O ----
    xTs = nc.dram_tensor("xTs", [M, TSH], dt.float32, kind="ExternalInput")
    xb = nc.dram_tensor("xb", [T, M], dt.bfloat16, kind="ExternalInput")
    wg = nc.dram_tensor("wg", [M, E], dt.float32, kind="ExternalInput")
    w1p = nc.dram_tensor("w1p", [DC, P, MC, P], dt.bfloat16, kind="ExternalInput")
    w2p = nc.dram_tensor("w2p", [P, DC, M], dt.bfloat16, kind="ExternalInput")
    b1v = nc.dram_tensor("b1v", [DFF], dt.float32, kind="ExternalInput")
    b2b = nc.dram_tensor("b2b", [P, M], dt.float32, kind="ExternalInput")
    ecol = nc.dram_tensor("ecol", [P, 1], dt.float32, kind="ExternalInput")
    eiota = nc.dram_tensor("eiota", [P, LT, E], dt.float32, kind="ExternalInput")
    toksf = nc.dram_tensor("toksf", [P, NT], dt.float32, kind="ExternalInput")
    triu = nc.dram_tensor("triu", [P, P], dt.float32, kind="ExternalInput")
    identf = nc.dram_tensor("identf", [P, P], dt.float32, kind="ExternalInput")
    identb = nc.dram_tensor("identb", [P, P], dt.bfloat16, kind="ExternalInput")
    pfill = nc.dram_tensor("pfill", [P, SCN, 2], dt.float32, kind="ExternalInput")
    outd = nc.dram_tensor("out", [T, M], dt.float32, kind="ExternalOutput")

    # ---- internal DRAM ----
    eg_loc = nc.dram_tensor("eg_loc", [TSH, 2], dt.float32)
    eg_all = nc.dram_tensor("eg_all", [T, 2], dt.float32, addr_space="Shared")
    igds = [nc.dram_tensor(f"igd{k}", [C, 2], dt.float32) for k in range(NIG)]

    with tile.TileContext(nc) as tc, ExitStack() as ctx:
        sb = ctx.enter_context(tc.tile_pool(name="sb", bufs=1))
        sbx = ctx.enter_context(tc.tile_pool(name="sbx", bufs=9))   # x stream
        sbw1 = ctx.enter_context(tc.tile_pool(name="sbw1", bufs=3))  # w1 stream
        sbg = ctx.enter_context(tc.tile_pool(name="sbg", bufs=3))   # gather tiles
        sbst = ctx.enter_context(tc.tile_pool(name="sbst", bufs=3))  # staging
        sbr = ctx.enter_context(tc.tile_pool(name="sbr", bufs=4))   # routing small

        # ---------- persistent tiles (small consts on sync queue) ----------
        wgt = sb.tile([P, MC * E], dt.float32)       # gate weights (mc, e)
        nc.sync.dma_start(wgt[:], wg[:].rearrange("(mc p) e -> p mc e", p=P))
        b1t = sb.tile([P, DC], dt.float32)           # b1 per-partition cols
        nc.sync.dma_start(b1t[:], b1v[:].rearrange("(d p) -> p d", p=P))
        ect = sb.tile([P, 1], dt.float32)
        nc.sync.dma_start(ect[:], ecol[:])
        eit = sb.tile([P, LT * E], dt.float32)
        nc.sync.dma_start(eit[:], eiota[:])
        tokf = sb.tile([P, NT], dt.float32)
        nc.sync.dma_start(tokf[:], toksf[:])
        trit = sb.tile([P, P], dt.float32)
        nc.sync.dma_start(trit[:], triu[:])
        idf = sb.tile([P, P], dt.float32)
        nc.sync.dma_start(idf[:], identf[:])
        idb = sb.tile([P, P], dt.bfloat16)
        nc.sync.dma_start(idb[:], identb[:])
        pft = sb.tile([P, SCN * 2], dt.float32)
        nc.sync.dma_start(pft[:], pfill[:])
        ones1 = sb.tile([1, P], dt.float32)
        nc.gpsimd.memset(ones1[:], 1.0)
        onescol = sb.tile([P, 1], dt.float32)
        nc.gpsimd.memset(onescol[:], 1.0)
        bigcol = sb.tile([P, NT], dt.float32)
        nc.gpsimd.memset(bigcol[:], BIG)
        nines = sb.tile([P, LT * E], dt.float32)
        nc.gpsimd.memset(nines[:], 9.0)
        # big tensors on the scalar (ACT) HWDGE queue so they don't block x
        b2t = sb.tile([P, M], dt.float32)
        nc.scalar.dma_start(b2t[:], b2b[:])
        w2t = sb.tile([P, DC * M], dt.bfloat16)      # resident w2 (d, m)
        if stage >= 5:
            for q in range(4):
                nc.scalar.dma_start(
                    w2t[:, q * 8 * M:(q + 1) * 8 * M],
                    w2p[:, q * 8:(q + 1) * 8, :])

        # prefill idx/gate stripes: BIG ids (skipped everywhere), zero gates
        for k in range(NIG):
            nc.sync.dma_start(
                igds[k][:].rearrange("(c p) two -> p c two", p=P), pft[:])

        eg_stk = sb.tile([P, LT * 2], dt.float32)    # local (eidx, gate) cols
        idx_t = sb.tile([P, SCN], dt.int32)
        gate_f = sb.tile([P, SCN], dt.float32)

        # ---------- phase A: sharded gating, batched routing ----------
        lg_stk = sb.tile([P, LT * E], dt.float32)
        with tc.tile_pool(name="psg", bufs=8, space="PSUM") as psg:
            xts = []
            for k in range(MC):
                xt = sbx.tile([P, TSH], dt.float32, tag="xt")
                nc.sync.dma_start(xt[:], xTs[k * P:(k + 1) * P, :])
                xts.append(xt)
            for ti in range(LT):
                pg = psg.tile([P, E], dt.float32, tag="pg")
                for k in range(MC):
                    nc.tensor.matmul(
                        pg[:], lhsT=xts[k][:, ti * P:(ti + 1) * P],
                        rhs=wgt[:, k * E:(k + 1) * E],
                        start=(k == 0), stop=(k == MC - 1))
                nc.vector.tensor_copy(lg_stk[:, ti * E:(ti + 1) * E], pg[:])
        lg3 = lg_stk[:].rearrange("p (ti e) -> p ti e", e=E)
        mx_stk = sb.tile([P, LT], dt.float32)
        nc.vector.tensor_reduce(
            out=mx_stk[:], in_=lg3, axis=mybir.AxisListType.X,
            op=mybir.AluOpType.max)
        mxb = mx_stk[:].rearrange("p (ti one) -> p ti one", one=1).to_broadcast([P, LT, E])
        ls = sb.tile([P, LT * E], dt.float32)
        nc.vector.tensor_tensor(
            out=ls[:].rearrange("p (ti e) -> p ti e", e=E), in0=lg3, in1=mxb,
            op=mybir.AluOpType.subtract)
        ex = sb.tile([P, LT * E], dt.float32)
        nc.scalar.activation(
            ex[:], ls[:], mybir.ActivationFunctionType.Exp)
        s_stk = sb.tile([P, LT], dt.float32)
        nc.vector.tensor_reduce(
            out=s_stk[:], in_=ex[:].rearrange("p (ti e) -> p ti e", e=E),
            axis=mybir.AxisListType.X, op=mybir.AluOpType.add)
        # gate into interleaved eg_stk col 2ti+1
        nc.vector.reciprocal(
            eg_stk[:].rearrange("p (ti two) -> p ti two", two=2)[:, :, 1:2],
            s_stk[:].rearrange("p (ti one) -> p ti one", one=1))
        # argmax with first-index tie-break: min over (onehot ? e : 9)
        oh = sb.tile([P, LT * E], dt.uint8)
        nc.vector.tensor_tensor(
            out=oh[:].rearrange("p (ti e) -> p ti e", e=E), in0=lg3, in1=mxb,
            op=mybir.AluOpType.is_equal)
        msk = sb.tile([P, LT * E], dt.float32)
        nc.vector.select(msk[:], oh[:], eit[:], nines[:])
        nc.vector.tensor_reduce(
            out=eg_stk[:].rearrange("p (ti two) -> p ti two", two=2)[:, :, 0:1],
            in_=msk[:].rearrange("p (ti e) -> p ti e", e=E),
            axis=mybir.AxisListType.X, op=mybir.AluOpType.min)

        # share routing metadata with all cores
        nc.sync.dma_start(
            eg_loc[:].rearrange("(t p) two -> p t two", p=P), eg_stk[:])
        nc.gpsimd.collective_compute(
            "AllGather", mybir.AluOpType.bypass,
            ins=[eg_loc[:]], outs=[eg_all[:]],
            replica_groups=[list(range(E))])

        if stage < 2:
            nc.compile()
            return nc

        # ---------- phase B: batched positions + token->slot inversion -----
        eidx_stk = sb.tile([P, NT], dt.float32)
        gate_stk = sb.tile([P, NT], dt.float32)
        nc.sync.dma_start(
            eidx_stk[:],
            eg_all[:].rearrange("(t p) two -> p t two", p=P)[:, :, 0:1])
        nc.sync.dma_start(
            gate_stk[:],
            eg_all[:].rearrange("(t p) two -> p t two", p=P)[:, :, 1:2])
        mine_stk = sb.tile([P, NT], dt.float32)
        nc.vector.tensor_scalar(
            out=mine_stk[:], in0=eidx_stk[:], scalar1=ect[:, 0:1], scalar2=None,
            op0=mybir.AluOpType.is_equal)

        with tc.tile_pool(name="ppb", bufs=1, space="PSUM") as ppb:
            # tile sums -> [1, NT] row via matmul + transpose
            pts = ppb.tile([64, 1], dt.float32, tag="pts")
            nc.tensor.matmul(pts[:], lhsT=mine_stk[:], rhs=onescol[:],
                             start=True, stop=True)
            tcol = sb.tile([64, 1], dt.float32)
            nc.vector.tensor_copy(tcol[:], pts[:])
            ptr = ppb.tile([1, 64], dt.float32, tag="ptr")
            nc.tensor.transpose(out=ptr[:], in_=tcol[:], identity=idf[:64, :64])
            tsums = sb.tile([1, NT], dt.float32)
            nc.vector.tensor_copy(tsums[:], ptr[:])
            zrow = sb.tile([1, NT], dt.float32)
            nc.gpsimd.memset(zrow[:], 0.0)
            incl = sb.tile([1, NT], dt.float32)
            nc.vector.tensor_tensor_scan(
                out=incl[:], data0=tsums[:], data1=zrow[:], initial=0.0,
                op0=mybir.AluOpType.add, op1=mybir.AluOpType.add)
            offs = sb.tile([1, NT], dt.float32)
            nc.vector.tensor_tensor(
                out=offs[:], in0=incl[:], in1=tsums[:],
                op=mybir.AluOpType.subtract)
            # batched positions for all 64 tiles
            pall = ppb.tile([P, NT], dt.float32, tag="pall")
            nc.tensor.matmul(pall[:], lhsT=trit[:], rhs=mine_stk[:],
                             start=True, stop=False)
            nc.tensor.matmul(pall[:], lhsT=ones1[:], rhs=offs[:],
                             start=False, stop=True)
            slotm1 = sb.tile([P, NT], dt.float32)
            nc.vector.tensor_scalar_add(slotm1[:], pall[:], -1.0)
        mu8 = sb.tile([P, NT], dt.uint8)
        nc.vector.tensor_scalar(
            out=mu8[:], in0=mine_stk[:], scalar1=0.5, scalar2=None,
            op0=mybir.AluOpType.is_gt)
        slotf = sb.tile([P, NT], dt.float32)
        nc.vector.select(slotf[:], mu8[:], slotm1[:], bigcol[:])
        sloti = sb.tile([P, NT], dt.int32)
        nc.vector.tensor_copy(sloti[:], slotf[:])
        # (token_id, gate) pairs; interleaved columns
        pairs = sb.tile([P, NT * 2], dt.float32)
        nc.vector.tensor_copy(
            pairs[:].rearrange("p (t two) -> p t two", two=2)[:, :, 0:1],
            tokf[:])
        nc.vector.tensor_copy(
            pairs[:].rearrange("p (t two) -> p t two", two=2)[:, :, 1:2],
            gate_stk[:])
        for t in range(NT):
            nc.gpsimd.indirect_dma_start(
                out=igds[t % NIG][:], out_offset=bass.IndirectOffsetOnAxis(
                    ap=sloti[:, t:t + 1], axis=0),
                in_=pairs[:, 2 * t:2 * t + 2], in_offset=None,
                bounds_check=C - 1, oob_is_err=False)
        # merge the stripes: each slot is filled in at most one stripe
        idxf = sb.tile([P, SCN], dt.float32)
        nc.sync.dma_start(
            idxf[:],
            igds[0][:].rearrange("(c p) two -> p c two", p=P)[:, :, 0:1])
        nc.sync.dma_start(
            gate_f[:],
            igds[0][:].rearrange("(c p) two -> p c two", p=P)[:, :, 1:2])
        for k in range(1, NIG):
            ikf = sbr.tile([P, SCN], dt.float32, tag="ikf")
            gkf = sbr.tile([P, SCN], dt.float32, tag="gkf")
            nc.sync.dma_start(
                ikf[:],
                igds[k][:].rearrange("(c p) two -> p c two", p=P)[:, :, 0:1])
            nc.sync.dma_start(
                gkf[:],
                igds[k][:].rearrange("(c p) two -> p c two", p=P)[:, :, 1:2])
            vk = sbr.tile([P, SCN], dt.uint8, tag="vk")
            nc.vector.tensor_scalar(
                out=vk[:], in0=ikf[:], scalar1=BIG * 0.5, scalar2=None,
                op0=mybir.AluOpType.is_lt)
            nc.vector.copy_predicated(idxf[:], vk[:], ikf[:])
            nc.vector.copy_predicated(gate_f[:], vk[:], gkf[:])
        nc.vector.tensor_copy(idx_t[:], idxf[:])

        if stage < 3:
            nc.compile()
            return nc

        # ---------- phases C/D/E per half ----------
        with (
            tc.tile_pool(name="pstr", bufs=2, space="PSUM") as pstr,
            tc.tile_pool(name="ps1", bufs=2, space="PSUM") as ps1,
            tc.tile_pool(name="ps2", bufs=2, space="PSUM") as ps2,
        ):
            for h in range(2):
                dispT = sb.tile([P, MC * HALF], dt.bfloat16, tag="dispT")
                hT = sb.tile([P, DC * HALF], dt.bfloat16, tag="hT")
                # dispatch: gather + transpose
                for s5 in range(5):
                    sc = h * 5 + s5
                    gx = sbg.tile([P, M], dt.bfloat16, tag="gx")
                    nc.gpsimd.memset(gx[:], 0.0)
                    nc.gpsimd.indirect_dma_start(
                        out=gx[:], out_offset=None, in_=xb[:],
                        in_offset=bass.IndirectOffsetOnAxis(
                            ap=idx_t[:, sc:sc + 1], axis=0),
                        bounds_check=T - 1, oob_is_err=False)
                    for mm in range(MC):
                        ptg = pstr.tile([P, P], dt.bfloat16, tag="ptg")
                        nc.tensor.transpose(
                            out=ptg[:], in_=gx[:, mm * P:(mm + 1) * P],
                            identity=idb[:])
                        nc.vector.tensor_copy(
                            dispT[:, mm * HALF + s5 * P:
                                  mm * HALF + (s5 + 1) * P],
                            ptg[:])
                # FFN1
                if stage >= 4:
                    for d in range(DC):
                        w1t = sbw1.tile([P, M], dt.bfloat16, tag="w1t")
                        nc.sync.dma_start(w1t[:], w1p[d])
                        pA = ps1.tile([P, 512], dt.float32, tag="pA")
                        pB = ps1.tile([P, P], dt.float32, tag="pB")
                        for mc in range(MC):
                            lhs = w1t[:, mc * P:(mc + 1) * P]
                            nc.tensor.matmul(
                                pA[:], lhsT=lhs,
                                rhs=dispT[:, mc * HALF:mc * HALF + 512],
                                start=(mc == 0), stop=(mc == MC - 1))
                            nc.tensor.matmul(
                                pB[:], lhsT=lhs,
                                rhs=dispT[:, mc * HALF + 512:(mc + 1) * HALF],
                                start=(mc == 0), stop=(mc == MC - 1))
                        nc.scalar.activation(
                            hT[:, d * HALF:d * HALF + 512], pA[:],
                            mybir.ActivationFunctionType.Relu,
                            bias=b1t[:, d:d + 1], scale=1.0)
                        nc.scalar.activation(
                            hT[:, d * HALF + 512:(d + 1) * HALF], pB[:],
                            mybir.ActivationFunctionType.Relu,
                            bias=b1t[:, d:d + 1], scale=1.0)
                # FFN2 + combine + scatter
                if stage >= 5:
                    for s5 in range(5):
                        sc = h * 5 + s5
                        st = sbst.tile([P, M], dt.float32, tag="st")
                        po0 = ps2.tile([P, 512], dt.float32, tag="po")
                        po1 = ps2.tile([P, 512], dt.float32, tag="po")
                        for d in range(DC):
                            lhs = hT[:, d * HALF + s5 * P:d * HALF + (s5 + 1) * P]
                            nc.tensor.matmul(
                                po0[:], lhsT=lhs,
                                rhs=w2t[:, d * M:d * M + 512],
                                start=(d == 0), stop=(d == DC - 1))
                            nc.tensor.matmul(
                                po1[:], lhsT=lhs,
                                rhs=w2t[:, d * M + 512:d * M + 1024],
                                start=(d == 0), stop=(d == DC - 1))
                        for mm, po in ((0, po0), (1, po1)):
                            nc.vector.tensor_tensor(
                                out=st[:, mm * 512:(mm + 1) * 512], in0=po[:],
                                in1=b2t[:, mm * 512:(mm + 1) * 512],
                                op=mybir.AluOpType.add)
                        nc.vector.tensor_scalar_mul(
                            st[:], st[:], gate_f[:, sc:sc + 1])
                        nc.gpsimd.indirect_dma_start(
                            out=outd[:], out_offset=bass.IndirectOffsetOnAxis(
                                ap=idx_t[:, sc:sc + 1], axis=0),
                            in_=st[:], in_offset=None,
                            bounds_check=T - 1, oob_is_err=False)

    nc.compile()
    return nc


def _prep_inputs(x, wg, w1, b1, w2, b2):
    bf16 = ml_dtypes.bfloat16
    tokens = np.ascontiguousarray(x.reshape(T, M)).astype(np.float32)
    xT = np.ascontiguousarray(tokens.T)
    xb = tokens.astype(bf16)
    wgf = np.ascontiguousarray(wg.astype(np.float32))
    toksf = np.arange(T, dtype=np.float32).reshape(NT, P).T.copy()
    eiota = np.broadcast_to(
        np.arange(E, dtype=np.float32), (P, LT, E)).copy()
    triu = np.triu(np.ones((P, P), dtype=np.float32))
    identf = np.eye(P, dtype=np.float32)
    identb = np.eye(P).astype(bf16)
    pfill = np.zeros((P, SCN, 2), dtype=np.float32)
    pfill[:, :, 0] = BIG
    in_maps = []
    for e in range(E):
        w1e = np.ascontiguousarray(w1[e]).astype(bf16)          # [M, DFF]
        w1pk = np.ascontiguousarray(
            w1e.reshape(MC, P, DC, P).transpose(2, 1, 0, 3))    # [DC,P,MC,P]
        w2e = np.ascontiguousarray(w2[e]).astype(bf16)          # [DFF, M]
        w2pk = np.ascontiguousarray(
            w2e.reshape(DC, P, M).transpose(1, 0, 2))           # [P,DC,M]
        in_maps.append({
            "xTs": np.ascontiguousarray(xT[:, e * TSH:(e + 1) * TSH]),
            "xb": xb, "wg": wgf,
            "w1p": w1pk, "w2p": w2pk,
            "b1v": np.ascontiguousarray(b1[e]).astype(np.float32),
            "b2b": np.tile(np.asarray(b2[e], dtype=np.float32), (P, 1)),
            "ecol": np.full((P, 1), float(e), dtype=np.float32),
            "eiota": eiota, "toksf": toksf, "triu": triu,
            "identf": identf, "identb": identb, "pfill": pfill,
        })
    return in_maps


def kernel(x, wg, w1, b1, w2, b2, _trace=False):
    if "nc" not in _CACHE:
        _CACHE["nc"] = _build_nc()
    nc = _CACHE["nc"]
    in_maps = _prep_inputs(
        np.asarray(x), np.asarray(wg), np.asarray(w1),
        np.asarray(b1), np.asarray(w2), np.asarray(b2))
    res = run_bass_kernel_spmd(nc, in_maps, list(range(E)), trace=_trace)
    _CACHE["last_results"] = res
    full = np.zeros((T, M), dtype=np.float32)
    for e in range(E):
        full += res.results[e]["out"]
    return full.reshape(B, S, M)


# revision 20
# speedup vs baseline: 1.3197x; 1.0174x over previous
"""MoE top-1 routing kernel for 8 TRN2 NeuronCores (expert parallelism).

Self-contained: takes full inputs, shards experts across 8 cores, returns the
full output (host sums the 8 disjoint per-expert partials).

v3 design:
- Gating token-sharded: core j computes fp32 logits for its 1024 tokens with
  batched whole-shard DVE routing (argmax via masked reduce-min, gate =
  1/sum(exp(l-max))), then one AllGather shares [T, 2] (expert_id, gate).
- Positions: one triangular matmul + one tensor_tensor_scan; token->slot
  inverted via 64 row-scatters of (token_id, gate) pairs striped over 4
  BIG-prefilled DRAM arrays (4 independent completion chains), merged with
  selects on load. OOB slots silently dropped => capacity drop for free.
- FFN in bf16, w2 resident, w1 streamed twice over two 640-slot halves,
  fused bias+ReLU on ACT, gate-scaled rows scattered into pre-zeroed output.
"""
import numpy as np
import ml_dtypes
from contextlib import ExitStack

import concourse.bass as bass
import concourse.tile as tile
from concourse import bacc, mybir
from concourse.bass_utils import run_bass_kernel_spmd

dt = mybir.dt

B, S, M, E, DFF = 4, 2048, 1024, 8, 4096
T = B * S                  # 8192 tokens
C = int(1.25 * T / E)      # 1280 capacity
P = 128
NT = T // P                # 64 token tiles
MC = M // P                # 8 m chunks
DC = DFF // P              # 32 dff chunks
SCN = C // P               # 10 slot chunks
HALF = C // 2              # 640 slots per half
TSH = T // E               # 1024 tokens per shard
LT = TSH // P              # 8 local tiles
NIG = 8                    # scatter stripe count
BIG = 1.0e9
BIGI = 2 ** 30

_CACHE = {}


def _build_nc(stage=5):
    nc = bacc.Bacc("TRN2", target_bir_lowering=False, debug=False)

    # ---- I/O ----
    xTs = nc.dram_tensor("xTs", [M, TSH], dt.float32, kind="ExternalInput")
    xb = nc.dram_tensor("xb", [T, M], dt.bfloat16, kind="ExternalInput")
    wg = nc.dram_tensor("wg", [M, E], dt.float32, kind="ExternalInput")
    w1p = nc.dram_tensor("w1p", [DC, P, MC, P], dt.bfloat16, kind="ExternalInput")
    w2p = nc.dram_tensor("w2p", [P, DC, M], dt.bfloat16, kind="ExternalInput")
    b1v = nc.dram_tensor("b1v", [DFF], dt.float32, kind="ExternalInput")
    b2b = nc.dram_tensor("b2b", [P, M], dt.float32, kind="ExternalInput")
    ecol = nc.dram_tensor("ecol", [P, 1], dt.float32, kind="ExternalInput")
    eiota = nc.dram_tensor("eiota", [P, LT, E], dt.float32, kind="ExternalInput")
    toksf = nc.dram_tensor("toksf", [P, NT], dt.float32, kind="ExternalInput")
    triu = nc.dram_tensor("triu", [P, P], dt.float32, kind="ExternalInput")
    identf = nc.dram_tensor("identf", [P, P], dt.float32, kind="ExternalInput")
    identb = nc.dram_tensor("identb", [P, P], dt.bfloat16, kind="ExternalInput")
    pfill = nc.dram_tensor("pfill", [P, SCN, 2], dt.float32, kind="ExternalInput")
    outd = nc.dram_tensor("out", [T, M], dt.float32, kind="ExternalOutput")

    # ---- internal DRAM ----
    eg_loc = nc.dram_tensor("eg_loc", [TSH, 2], dt.float32)
    eg_all = nc.dram_tensor("eg_all", [T, 2], dt.float32, addr_space="Shared")
    igds = [nc.dram_tensor(f"igd{k}", [C, 2], dt.float32) for k in range(NIG)]

    with tile.TileContext(nc) as tc, ExitStack() as ctx:
        sb = ctx.enter_context(tc.tile_pool(name="sb", bufs=1))
        sbx = ctx.enter_context(tc.tile_pool(name="sbx", bufs=9))   # x stream
        sbw1 = ctx.enter_context(tc.tile_pool(name="sbw1", bufs=3))  # w1 stream
        sbg = ctx.enter_context(tc.tile_pool(name="sbg", bufs=3))   # gather tiles
        sbst = ctx.enter_context(tc.tile_pool(name="sbst", bufs=3))  # staging
        sbr = ctx.enter_context(tc.tile_pool(name="sbr", bufs=4))   # routing small

        # ---------- persistent tiles (small consts on sync queue) ----------
        wgt = sb.tile([P, MC * E], dt.float32)       # gate weights (mc, e)
        nc.sync.dma_start(wgt[:], wg[:].rearrange("(mc p) e -> p mc e", p=P))
        b1t = sb.tile([P, DC], dt.float32)           # b1 per-partition cols
        nc.sync.dma_start(b1t[:], b1v[:].rearrange("(d p) -> p d", p=P))
        ect = sb.tile([P, 1], dt.float32)
        nc.sync.dma_start(ect[:], ecol[:])
        eit = sb.tile([P, LT * E], dt.float32)
        nc.sync.dma_start(eit[:], eiota[:])
        tokf = sb.tile([P, NT], dt.float32)
        nc.sync.dma_start(tokf[:], toksf[:])
        trit = sb.tile([P, P], dt.float32)
        nc.sync.dma_start(trit[:], triu[:])
        idf = sb.tile([P, P], dt.float32)
        nc.sync.dma_start(idf[:], identf[:])
        idb = sb.tile([P, P], dt.bfloat16)
        nc.sync.dma_start(idb[:], identb[:])
        pft = sb.tile([P, SCN * 2], dt.float32)
        nc.sync.dma_start(pft[:], pfill[:])
        ones1 = sb.tile([1, P], dt.float32)
        nc.gpsimd.memset(ones1[:], 1.0)
        onescol = sb.tile([P, 1], dt.float32)
        nc.gpsimd.memset(onescol[:], 1.0)
        bigcol = sb.tile([P, NT], dt.float32)
        nc.gpsimd.memset(bigcol[:], BIG)
        nines = sb.tile([P, LT * E], dt.float32)
        nc.gpsimd.memset(nines[:], 9.0)
        # big tensors on the scalar (ACT) HWDGE queue so they don't block x
        b2t = sb.tile([P, M], dt.float32)
        nc.scalar.dma_start(b2t[:], b2b[:])
        w2t = sb.tile([P, DC * M], dt.bfloat16)      # resident w2 (d, m)
        if stage >= 5:
            for q in range(4):
                nc.scalar.dma_start(
                    w2t[:, q * 8 * M:(q + 1) * 8 * M],
                    w2p[:, q * 8:(q + 1) * 8, :])

        # prefill idx/gate stripes: BIG ids (skipped everywhere), zero gates
        for k in range(NIG):
            nc.scalar.dma_start(
                igds[k][:].rearrange("(c p) two -> p c two", p=P), pft[:])

        eg_stk = sb.tile([P, LT * 2], dt.float32)    # local (eidx, gate) cols
        idx_t = sb.tile([P, SCN], dt.int32)
        gate_f = sb.tile([P, SCN], dt.float32)

        # ---------- phase A: sharded gating, batched routing ----------
        # wg is the stationary (8 tiny weight loads); logits come out
        # transposed [E, tok] and are PE-transposed back per 128-token tile.
        lg_stk = sb.tile([P, LT * E], dt.float32)
        with tc.tile_pool(name="psg", bufs=4, space="PSUM") as psg:
            xts = []
            for k in range(MC):
                xt = sbx.tile([P, TSH], dt.float32, tag="xt")
                nc.sync.dma_start(xt[:], xTs[k * P:(k + 1) * P, :])
                xts.append(xt)
            lgT = sb.tile([8, TSH], dt.float32)
            for blk in range(TSH // 512):
                pl = psg.tile([8, 512], dt.float32, tag="pl")
                for k in range(MC):
                    nc.tensor.matmul(
                        pl[:], lhsT=wgt[:, k * E:(k + 1) * E],
                        rhs=xts[k][:, blk * 512:(blk + 1) * 512],
                        start=(k == 0), stop=(k == MC - 1))
                nc.vector.tensor_copy(lgT[:, blk * 512:(blk + 1) * 512], pl[:])
            for ti in range(LT):
                pq = psg.tile([P, E], dt.float32, tag="pq")
                nc.tensor.transpose(
                    out=pq[:], in_=lgT[:, ti * P:(ti + 1) * P],
                    identity=idf[:8, :8])
                nc.vector.tensor_copy(lg_stk[:, ti * E:(ti + 1) * E], pq[:])
        lg3 = lg_stk[:].rearrange("p (ti e) -> p ti e", e=E)
        mx_stk = sb.tile([P, LT], dt.float32)
        nc.vector.tensor_reduce(
            out=mx_stk[:], in_=lg3, axis=mybir.AxisListType.X,
            op=mybir.AluOpType.max)
        mxb = mx_stk[:].rearrange("p (ti one) -> p ti one", one=1).to_broadcast([P, LT, E])
        ls = sb.tile([P, LT * E], dt.float32)
        nc.vector.tensor_tensor(
            out=ls[:].rearrange("p (ti e) -> p ti e", e=E), in0=lg3, in1=mxb,
            op=mybir.AluOpType.subtract)
        ex = sb.tile([P, LT * E], dt.float32)
        nc.scalar.activation(
            ex[:], ls[:], mybir.ActivationFunctionType.Exp)
        s_stk = sb.tile([P, LT], dt.float32)
        nc.vector.tensor_reduce(
            out=s_stk[:], in_=ex[:].rearrange("p (ti e) -> p ti e", e=E),
            axis=mybir.AxisListType.X, op=mybir.AluOpType.add)
        # gate into interleaved eg_stk col 2ti+1
        nc.vector.reciprocal(
            eg_stk[:].rearrange("p (ti two) -> p ti two", two=2)[:, :, 1:2],
            s_stk[:].rearrange("p (ti one) -> p ti one", one=1))
        # argmax with first-index tie-break: min over (onehot ? e : 9)
        oh = sb.tile([P, LT * E], dt.uint8)
        nc.vector.tensor_tensor(
            out=oh[:].rearrange("p (ti e) -> p ti e", e=E), in0=lg3, in1=mxb,
            op=mybir.AluOpType.is_equal)
        msk = sb.tile([P, LT * E], dt.float32)
        nc.vector.select(msk[:], oh[:], eit[:], nines[:])
        nc.vector.tensor_reduce(
            out=eg_stk[:].rearrange("p (ti two) -> p ti two", two=2)[:, :, 0:1],
            in_=msk[:].rearrange("p (ti e) -> p ti e", e=E),
            axis=mybir.AxisListType.X, op=mybir.AluOpType.min)

        # share routing metadata with all cores
        nc.sync.dma_start(
            eg_loc[:].rearrange("(t p) two -> p t two", p=P), eg_stk[:])
        nc.gpsimd.collective_compute(
            "AllGather", mybir.AluOpType.bypass,
            ins=[eg_loc[:]], outs=[eg_all[:]],
            replica_groups=[list(range(E))])

        if stage < 2:
            nc.compile()
            return nc

        # ---------- phase B: batched positions + token->slot inversion -----
        eidx_stk = sb.tile([P, NT], dt.float32)
        gate_stk = sb.tile([P, NT], dt.float32)
        nc.sync.dma_start(
            eidx_stk[:],
            eg_all[:].rearrange("(t p) two -> p t two", p=P)[:, :, 0:1])
        nc.sync.dma_start(
            gate_stk[:],
            eg_all[:].rearrange("(t p) two -> p t two", p=P)[:, :, 1:2])
        mine_stk = sb.tile([P, NT], dt.float32)
        nc.vector.tensor_scalar(
            out=mine_stk[:], in0=eidx_stk[:], scalar1=ect[:, 0:1], scalar2=None,
            op0=mybir.AluOpType.is_equal)

        with tc.tile_pool(name="ppb", bufs=1, space="PSUM") as ppb:
            # tile sums -> [1, NT] row via matmul + transpose
            pts = ppb.tile([64, 1], dt.float32, tag="pts")
            nc.tensor.matmul(pts[:], lhsT=mine_stk[:], rhs=onescol[:],
                             start=True, stop=True)
            tcol = sb.tile([64, 1], dt.float32)
            nc.vector.tensor_copy(tcol[:], pts[:])
            ptr = ppb.tile([1, 64], dt.float32, tag="ptr")
            nc.tensor.transpose(out=ptr[:], in_=tcol[:], identity=idf[:64, :64])
            tsums = sb.tile([1, NT], dt.float32)
            nc.vector.tensor_copy(tsums[:], ptr[:])
            zrow = sb.tile([1, NT], dt.float32)
            nc.gpsimd.memset(zrow[:], 0.0)
            incl = sb.tile([1, NT], dt.float32)
            nc.vector.tensor_tensor_scan(
                out=incl[:], data0=tsums[:], data1=zrow[:], initial=0.0,
                op0=mybir.AluOpType.add, op1=mybir.AluOpType.add)
            offs = sb.tile([1, NT], dt.float32)
            nc.vector.tensor_tensor(
                out=offs[:], in0=incl[:], in1=tsums[:],
                op=mybir.AluOpType.subtract)
            # batched positions for all 64 tiles
            pall = ppb.tile([P, NT], dt.float32, tag="pall")
            nc.tensor.matmul(pall[:], lhsT=trit[:], rhs=mine_stk[:],
                             start=True, stop=False)
            nc.tensor.matmul(pall[:], lhsT=ones1[:], rhs=offs[:],
                             start=False, stop=True)
            slotm1 = sb.tile([P, NT], dt.float32)
            nc.vector.tensor_scalar_add(slotm1[:], pall[:], -1.0)
        mu8 = sb.tile([P, NT], dt.uint8)
        nc.vector.tensor_scalar(
            out=mu8[:], in0=mine_stk[:], scalar1=0.5, scalar2=None,
            op0=mybir.AluOpType.is_gt)
        slotf = sb.tile([P, NT], dt.float32)
        nc.vector.select(slotf[:], mu8[:], slotm1[:], bigcol[:])
        sloti = sb.tile([P, NT], dt.int32)
        nc.vector.tensor_copy(sloti[:], slotf[:])
        # (token_id, gate) pairs; interleaved columns
        pairs = sb.tile([P, NT * 2], dt.float32)
        nc.vector.tensor_copy(
            pairs[:].rearrange("p (t two) -> p t two", two=2)[:, :, 0:1],
            tokf[:])
        nc.vector.tensor_copy(
            pairs[:].rearrange("p (t two) -> p t two", two=2)[:, :, 1:2],
            gate_stk[:])
        for t in range(NT):
            nc.gpsimd.indirect_dma_start(
                out=igds[t % NIG][:], out_offset=bass.IndirectOffsetOnAxis(
                    ap=sloti[:, t:t + 1], axis=0),
                in_=pairs[:, 2 * t:2 * t + 2], in_offset=None,
                bounds_check=C - 1, oob_is_err=False)
        # merge the stripes: each slot is filled in at most one stripe
        idxf = sb.tile([P, SCN], dt.float32)
        nc.sync.dma_start(
            idxf[:],
            igds[0][:].rearrange("(c p) two -> p c two", p=P)[:, :, 0:1])
        nc.sync.dma_start(
            gate_f[:],
            igds[0][:].rearrange("(c p) two -> p c two", p=P)[:, :, 1:2])
        for k in range(1, NIG):
            ikf = sbr.tile([P, SCN], dt.float32, tag="ikf")
            gkf = sbr.tile([P, SCN], dt.float32, tag="gkf")
            nc.sync.dma_start(
                ikf[:],
                igds[k][:].rearrange("(c p) two -> p c two", p=P)[:, :, 0:1])
            nc.sync.dma_start(
                gkf[:],
                igds[k][:].rearrange("(c p) two -> p c two", p=P)[:, :, 1:2])
            vk = sbr.tile([P, SCN], dt.uint8, tag="vk")
            nc.vector.tensor_scalar(
                out=vk[:], in0=ikf[:], scalar1=BIG * 0.5, scalar2=None,
                op0=mybir.AluOpType.is_lt)
            nc.vector.copy_predicated(idxf[:], vk[:], ikf[:])
            nc.vector.copy_predicated(gate_f[:], vk[:], gkf[:])
        nc.vector.tensor_copy(idx_t[:], idxf[:])

        if stage < 3:
            nc.compile()
            return nc

        # ---------- phases C/D/E per half ----------
        with (
            tc.tile_pool(name="pstr", bufs=2, space="PSUM") as pstr,
            tc.tile_pool(name="ps1", bufs=2, space="PSUM") as ps1,
            tc.tile_pool(name="ps2", bufs=2, space="PSUM") as ps2,
        ):
            for h in range(2):
                dispT = sb.tile([P, MC * HALF], dt.bfloat16, tag="dispT")
                hT = sb.tile([P, DC * HALF], dt.bfloat16, tag="hT")
                # dispatch: gather + transpose
                for s5 in range(5):
                    sc = h * 5 + s5
                    gx = sbg.tile([P, M], dt.bfloat16, tag="gx")
                    nc.gpsimd.memset(gx[:], 0.0)
                    nc.gpsimd.indirect_dma_start(
                        out=gx[:], out_offset=None, in_=xb[:],
                        in_offset=bass.IndirectOffsetOnAxis(
                            ap=idx_t[:, sc:sc + 1], axis=0),
                        bounds_check=T - 1, oob_is_err=False)
                    for mm in range(MC):
                        ptg = pstr.tile([P, P], dt.bfloat16, tag="ptg")
                        nc.tensor.transpose(
                            out=ptg[:], in_=gx[:, mm * P:(mm + 1) * P],
                            identity=idb[:])
                        nc.vector.tensor_copy(
                            dispT[:, mm * HALF + s5 * P:
                                  mm * HALF + (s5 + 1) * P],
                            ptg[:])
                # FFN1
                if stage >= 4:
                    for d in range(DC):
                        w1t = sbw1.tile([P, M], dt.bfloat16, tag="w1t")
                        nc.sync.dma_start(w1t[:], w1p[d])
                        pA = ps1.tile([P, 512], dt.float32, tag="pA")
                        pB = ps1.tile([P, P], dt.float32, tag="pB")
                        for mc in range(MC):
                            lhs = w1t[:, mc * P:(mc + 1) * P]
                            nc.tensor.matmul(
                                pA[:], lhsT=lhs,
                                rhs=dispT[:, mc * HALF:mc * HALF + 512],
                                start=(mc == 0), stop=(mc == MC - 1))
                            nc.tensor.matmul(
                                pB[:], lhsT=lhs,
                                rhs=dispT[:, mc * HALF + 512:(mc + 1) * HALF],
                                start=(mc == 0), stop=(mc == MC - 1))
                        nc.scalar.activation(
                            hT[:, d * HALF:d * HALF + 512], pA[:],
                            mybir.ActivationFunctionType.Relu,
                            bias=b1t[:, d:d + 1], scale=1.0)
                        nc.scalar.activation(
                            hT[:, d * HALF + 512:(d + 1) * HALF], pB[:],
                            mybir.ActivationFunctionType.Relu,
                            bias=b1t[:, d:d + 1], scale=1.0)
                # FFN2 + combine + scatter
                if stage >= 5:
                    for s5 in range(5):
                        sc = h * 5 + s5
                        st = sbst.tile([P, M], dt.float32, tag="st")
                        po0 = ps2.tile([P, 512], dt.float32, tag="po")
                        po1 = ps2.tile([P, 512], dt.float32, tag="po")
                        for d in range(DC):
                            lhs = hT[:, d * HALF + s5 * P:d * HALF + (s5 + 1) * P]
                            nc.tensor.matmul(
                                po0[:], lhsT=lhs,
                                rhs=w2t[:, d * M:d * M + 512],
                                start=(d == 0), stop=(d == DC - 1))
                            nc.tensor.matmul(
                                po1[:], lhsT=lhs,
                                rhs=w2t[:, d * M + 512:d * M + 1024],
                                start=(d == 0), stop=(d == DC - 1))
                        for mm, po in ((0, po0), (1, po1)):
                            nc.vector.tensor_tensor(
                                out=st[:, mm * 512:(mm + 1) * 512], in0=po[:],
                                in1=b2t[:, mm * 512:(mm + 1) * 512],
                                op=mybir.AluOpType.add)
                        nc.vector.tensor_scalar_mul(
                            st[:], st[:], gate_f[:, sc:sc + 1])
                        nc.gpsimd.indirect_dma_start(
                            out=outd[:], out_offset=bass.IndirectOffsetOnAxis(
                                ap=idx_t[:, sc:sc + 1], axis=0),
                            in_=st[:], in_offset=None,
                            bounds_check=T - 1, oob_is_err=False)

    nc.compile()
    return nc


def _prep_inputs(x, wg, w1, b1, w2, b2):
    bf16 = ml_dtypes.bfloat16
    tokens = np.ascontiguousarray(x.reshape(T, M)).astype(np.float32)
    xT = np.ascontiguousarray(tokens.T)
    xb = tokens.astype(bf16)
    wgf = np.ascontiguousarray(wg.astype(np.float32))
    toksf = np.arange(T, dtype=np.float32).reshape(NT, P).T.copy()
    eiota = np.broadcast_to(
        np.arange(E, dtype=np.float32), (P, LT, E)).copy()
    triu = np.triu(np.ones((P, P), dtype=np.float32))
    identf = np.eye(P, dtype=np.float32)
    identb = np.eye(P).astype(bf16)
    pfill = np.zeros((P, SCN, 2), dtype=np.float32)
    pfill[:, :, 0] = BIG
    in_maps = []
    for e in range(E):
        w1e = np.ascontiguousarray(w1[e]).astype(bf16)          # [M, DFF]
        w1pk = np.ascontiguousarray(
            w1e.reshape(MC, P, DC, P).transpose(2, 1, 0, 3))    # [DC,P,MC,P]
        w2e = np.ascontiguousarray(w2[e]).astype(bf16)          # [DFF, M]
        w2pk = np.ascontiguousarray(
            w2e.reshape(DC, P, M).transpose(1, 0, 2))           # [P,DC,M]
        in_maps.append({
            "xTs": np.ascontiguousarray(xT[:, e * TSH:(e + 1) * TSH]),
            "xb": xb, "wg": wgf,
            "w1p": w1pk, "w2p": w2pk,
            "b1v": np.ascontiguousarray(b1[e]).astype(np.float32),
            "b2b": np.tile(np.asarray(b2[e], dtype=np.float32), (P, 1)),
            "ecol": np.full((P, 1), float(e), dtype=np.float32),
            "eiota": eiota, "toksf": toksf, "triu": triu,
            "identf": identf, "identb": identb, "pfill": pfill,
        })
    return in_maps


def kernel(x, wg, w1, b1, w2, b2, _trace=False):
    if "nc" not in _CACHE:
        _CACHE["nc"] = _build_nc()
    nc = _CACHE["nc"]
    in_maps = _prep_inputs(
        np.asarray(x), np.asarray(wg), np.asarray(w1),
        np.asarray(b1), np.asarray(w2), np.asarray(b2))
    res = run_bass_kernel_spmd(nc, in_maps, list(range(E)), trace=_trace)
    _CACHE["last_results"] = res
    full = np.zeros((T, M), dtype=np.float32)
    for e in range(E):
        full += res.results[e]["out"]
    return full.reshape(B, S, M)
